# revision 21
# baseline (speedup 1.0000x reference)
"""CUR-DeepSeek-MoE Trainium2 kernel.

Strategy: token-parallel over 8 NeuronCores. Each core processes 512 of the
4096 tokens and reads all weights (replicated). The routed-expert sum is
computed dense-masked (every expert processes the core's 512 tokens, scaled by
the top-4 combine weight, which is zero for non-routed tokens) — numerically
identical to gather/scatter routing. Matmuls run as float32r (full PE rate at
free-dim >= 256, ~tf32 precision). No collectives.

Layout convention: activations live as [feature(part), token(free)] so every
GEMM contracts over partitions; nn.Linear weights [out,in] are transposed on
the PE (128x128 blocks via identity matmul) into [in(part), out(free)].
The final down-projection runs h-chunk-major producing y as [h, t], which is
transposed back to [t, h] before the DMA out.
"""

import os
import numpy as np
from contextlib import ExitStack

import concourse.bass as bass
import concourse.mybir as mybir
import concourse.tile as tile
from concourse import bacc
from concourse.bass_utils import run_bass_kernel_spmd
from concourse.masks import make_identity

F32 = mybir.dt.float32
F32R = mybir.dt.float32r
AF = mybir.ActivationFunctionType
ALU = mybir.AluOpType
AX = mybir.AxisListType

H = 2048
I = 1408
E = 32
RG = 128
SI = 2816
NCORES = 8
TFULL = 4096
TC = TFULL // NCORES          # 512 tokens per core
P = 128
HCN = H // P                  # 16
ICN = I // P                  # 11
SICN = SI // P                # 22
TCN = TC // P                 # 4


def _r(ap):
    return ap.bitcast(F32R)


class _B:
    """Emission helpers bound to one TileContext."""

    def __init__(self, nc):
        self.nc = nc
        self._ctr = 0

    def copy(self, out, in_):
        """PSUM->SBUF copy alternating scalar/vector engines."""
        self._ctr += 1
        if self._ctr % 2 == 0:
            self.nc.scalar.copy(out, in_)
        else:
            self.nc.vector.tensor_copy(out, in_)

    def transpose_pack(self, ps_pool, dst, srcs, identity, tag="tp"):
        """PE-transpose [p,f] SBUF blocks; pack outputs ([f,p]) along dst's
        free dim. Groups <=512 output floats per PSUM bank, one copy/bank."""
        nc = self.nc
        off = 0
        k = 0
        n = len(srcs)
        while k < n:
            width = 0
            take = 0
            while k + take < n:
                w = srcs[k + take].shape[0]
                if width + w > 512:
                    break
                width += w
                take += 1
            outp = srcs[k].shape[1]
            ps = ps_pool.tile([P, 512], F32, tag=tag)
            w0 = 0
            for s in srcs[k:k + take]:
                pw, fp = s.shape[0], s.shape[1]
                nc.tensor.transpose(ps[:fp, w0:w0 + pw], s, identity[:pw, :pw])
                w0 += pw
            self.copy(dst[:outp, off:off + width], ps[:outp, :width])
            off += width
            k += take


def build(nc):
    ap = {}
    specs = {
        "x": [TC, H], "gate_w": [E, H],
        "Rg": [RG, H], "Ru": [RG, H], "Rd": [RG, I],
        "Ug": [E, RG, RG], "Cg": [E, I, RG], "Uu": [E, RG, RG],
        "Cu": [E, I, RG], "Ud": [E, RG, RG], "Cd": [E, H, RG],
        "s_Rg": [RG, H], "s_Ug": [RG, RG], "s_Cg": [SI, RG],
        "s_Ru": [RG, H], "s_Uu": [RG, RG], "s_Cu": [SI, RG],
        "s_Rd": [RG, SI], "s_Ud": [RG, RG], "s_Cd": [H, RG],
    }
    for name, shape in specs.items():
        ap[name] = nc.dram_tensor(name, shape, F32, kind="ExternalInput").ap()
    y_dram = nc.dram_tensor("y", [TC, H], F32, kind="ExternalOutput").ap()
    we_dram = nc.dram_tensor("we_scratch", [E, TC], F32).ap()

    b = _B(nc)
    with tile.TileContext(nc) as tc, ExitStack() as ctx:
        res = ctx.enter_context(tc.tile_pool(name="res", bufs=1))

        identity = res.tile([P, P], F32, tag="ident")
        make_identity(nc, identity[:])

        # Resident across phases:
        rgT = res.tile([P, TC], F32R, tag="rgT")      # [rg, t]
        ruT = res.tile([P, TC], F32R, tag="ruT")
        srgT = res.tile([P, TC], F32R, tag="srgT")
        sruT = res.tile([P, TC], F32R, tag="sruT")
        weT = res.tile([E, TC], F32, tag="weT")      # [e, t] combine weights
        RdT = res.tile([P, I], F32R, tag="RdT")       # [i, rd]
        dT_all = res.tile([P, (E + 1) * TC], F32R, tag="dT_all")  # [rd', e*t]
        ones1 = res.tile([1, P], F32, tag="ones1")
        nc.gpsimd.memset(ones1[:], 1.0)

        # ================= Phase 0: prologue =================
        with ExitStack() as pctx:
            pro = pctx.enter_context(tc.tile_pool(name="pro", bufs=1))
            pst = pctx.enter_context(tc.tile_pool(name="pro_st", bufs=2))
            pps = pctx.enter_context(tc.tile_pool(name="pro_ps", bufs=2,
                                                  space="PSUM"))
            pp1 = pctx.enter_context(tc.tile_pool(name="pro_ps1", bufs=1,
                                                  space="PSUM"))

            # x shard -> xT, token-tile-major: [h(part), t*H + hc*P + hh]
            xT = pro.tile([P, TCN * H], F32R, tag="xT")
            for t in range(TCN):
                xs = pst.tile([P, H], F32, tag="xs")
                nc.sync.dma_start(xs[:], ap["x"][t * P:(t + 1) * P, :])
                b.transpose_pack(
                    pps, xT[:, t * H:(t + 1) * H],
                    [xs[:, hc * P:(hc + 1) * P] for hc in range(HCN)],
                    identity)
            xT_r = xT[:].rearrange("p (t h) -> p t h", h=H)

            def xT_hc(hc):
                # [h128(part), (t, 128 tokens)] strided rhs, N = TC
                return xT_r[:, :, hc * P:(hc + 1) * P]

            # gate_w -> gate_T blocks [h, e] per hc
            gate_nat = pst.tile([E, H], F32, tag="gate_nat")
            nc.sync.dma_start(gate_nat[:], ap["gate_w"][:])
            gate_T = pro.tile([P, HCN * E], F32R, tag="gate_T")
            b.transpose_pack(
                pps, gate_T,
                [gate_nat[:, hc * P:(hc + 1) * P] for hc in range(HCN)],
                identity)

            # Rd -> RdT [i, rd]
            rd_nat = pst.tile([P, H], F32, tag="r_nat")
            nc.sync.dma_start(rd_nat[:, :I], ap["Rd"][:])
            b.transpose_pack(
                pps, RdT,
                [rd_nat[:, ic * P:(ic + 1) * P] for ic in range(ICN)],
                identity)

            # R projections, streamed: transpose then accumulate rg et al.
            for name, dstT in (("Rg", rgT), ("Ru", ruT),
                               ("s_Rg", srgT), ("s_Ru", sruT)):
                nat = pst.tile([P, H], F32, tag="r_nat")
                nc.sync.dma_start(nat[:], ap[name][:])
                rt = pst.tile([P, H], F32R, tag="rT")
                b.transpose_pack(
                    pps, rt,
                    [nat[:, hc * P:(hc + 1) * P] for hc in range(HCN)],
                    identity)
                acc = pp1.tile([P, TC], F32, tag="acc")
                for hc in range(HCN):
                    nc.tensor.matmul(
                        acc[:], rt[:, hc * P:(hc + 1) * P],
                        xT_hc(hc),
                        start=(hc == 0), stop=(hc == HCN - 1))
                b.copy(dstT[:], acc[:])

            # gate logits + top-4 combine weights per token tile
            zeros = pro.tile([P, E], F32, tag="zeros")
            nc.gpsimd.memset(zeros[:], 0.0)
            we_sb = []
            for t in range(TCN):
                lg = pp1.tile([P, E], F32, tag="lg")
                for hc in range(HCN):
                    nc.tensor.matmul(
                        lg[:], xT_r[:, t, hc * P:(hc + 1) * P],
                        gate_T[:, hc * E:(hc + 1) * E],
                        start=(hc == 0), stop=(hc == HCN - 1))
                nmax = pro.tile([P, 1], F32, tag="nmax")
                nc.vector.reduce_max(nmax[:], lg[:], AX.X, negate=True)
                p = pro.tile([P, E], F32, tag=f"p{t}")
                nc.scalar.activation(p[:], lg[:], AF.Exp, bias=nmax[:])
                pc = pro.tile([P, E], F32, tag="pc")
                nc.vector.tensor_copy(pc[:], p[:])
                mx = pro.tile([P, 1], F32, tag="mx")
                msk = pro.tile([P, E], F32, tag="msk")
                msk_i = pro.tile([P, E], mybir.dt.uint8, tag="msk_i")
                for _ in range(3):
                    nc.vector.reduce_max(mx[:], pc[:], AX.X)
                    nc.vector.tensor_scalar(msk_i[:], pc[:], mx[:], None,
                                            ALU.is_equal)
                    nc.vector.copy_predicated(pc[:], msk_i[:], zeros[:])
                nc.vector.reduce_max(mx[:], pc[:], AX.X)  # 4th largest
                nc.vector.tensor_scalar(msk[:], p[:], mx[:], None, ALU.is_ge)
                wu = pro.tile([P, E], F32, tag="wu")
                nc.vector.tensor_tensor(wu[:], p[:], msk[:], ALU.mult)
                den = pro.tile([P, 1], F32, tag="den")
                nc.vector.reduce_sum(den[:], wu[:], AX.X)
                rec = pro.tile([P, 1], F32, tag="rec")
                nc.vector.reciprocal(rec[:], den[:])
                we_t = pro.tile([P, E], F32, tag=f"we{t}")
                nc.vector.tensor_scalar(we_t[:], wu[:], rec[:], None, ALU.mult)
                we_sb.append(we_t)
            b.transpose_pack(pps, weT, [w[:] for w in we_sb], identity)
            nc.sync.dma_start(we_dram[:], weT[:])

        # ================= Phase 1: shared experts =================
        with ExitStack() as sctx:
            ssb = sctx.enter_context(tc.tile_pool(name="sh_sb", bufs=1))
            sst = sctx.enter_context(tc.tile_pool(name="sh_st", bufs=2))
            sps = sctx.enter_context(tc.tile_pool(name="sh_ps", bufs=2,
                                                  space="PSUM"))
            sp1 = sctx.enter_context(tc.tile_pool(name="sh_ps1", bufs=1,
                                                  space="PSUM"))

            sut_st = sst.tile([P, 3 * P], F32, tag="su_st")
            for j, name in enumerate(("s_Ug", "s_Uu", "s_Ud")):
                nc.sync.dma_start(sut_st[:, j * P:(j + 1) * P], ap[name][:])
            sUT = ssb.tile([P, 3 * P], F32R, tag="sUT")
            b.transpose_pack(sps, sUT,
                             [sut_st[:, j * P:(j + 1) * P] for j in range(3)],
                             identity)

            s_CdT = res.tile([P, H], F32R, tag="s_CdT")
            wT = {}
            for name, blocks in (("s_Cg", SICN), ("s_Cu", SICN),
                                 ("s_Rd", SICN), ("s_Cd", HCN)):
                st = sst.tile([P, SICN * P], F32, tag="s_wide_st")
                if name == "s_Rd":
                    nc.sync.dma_start(st[:, :SI], ap[name][:])
                else:
                    nc.sync.dma_start(
                        st[:, :blocks * P].rearrange("p (n r) -> p n r", r=P),
                        ap[name].rearrange("(n p) r -> p n r", p=P))
                srcs = [st[:, ic * P:(ic + 1) * P] for ic in range(blocks)]
                if name == "s_Cd":
                    wT[name] = s_CdT
                else:
                    wT[name] = ssb.tile([P, blocks * P], F32R, tag=f"{name}_T",
                                        name=f"{name}_T")
                b.transpose_pack(sps, wT[name], srcs, identity)

            gp_ps = sp1.tile([P, TC], F32, tag="gp")
            nc.tensor.matmul(gp_ps[:], sUT[:, 0:P], srgT[:])
            g_pre = ssb.tile([P, TC], F32R, tag="g_pre")
            nc.scalar.copy(g_pre[:], gp_ps[:])
            up_ps = sp1.tile([P, TC], F32, tag="gp")
            nc.tensor.matmul(up_ps[:], sUT[:, P:2 * P], sruT[:])
            u_pre = ssb.tile([P, TC], F32R, tag="u_pre")
            nc.vector.tensor_copy(u_pre[:], up_ps[:])

            rd_ps = sp1.tile([P, TC], F32, tag="rd")
            for ic in range(SICN):
                g_ps = sps.tile([P, TC], F32, tag="g")
                nc.tensor.matmul(g_ps[:], wT["s_Cg"][:, ic * P:(ic + 1) * P],
                                 g_pre[:])
                u_ps = sps.tile([P, TC], F32, tag="u")
                nc.tensor.matmul(u_ps[:], wT["s_Cu"][:, ic * P:(ic + 1) * P],
                                 u_pre[:])
                g_sil = sst.tile([P, TC], F32, tag="g_sil")
                nc.scalar.activation(g_sil[:], g_ps[:], AF.Silu)
                m = sst.tile([P, TC], F32R, tag="m")
                nc.vector.tensor_tensor(m[:], g_sil[:], u_ps[:], ALU.mult)
                nc.tensor.matmul(rd_ps[:], wT["s_Rd"][:, ic * P:(ic + 1) * P],
                                 m[:], start=(ic == 0), stop=(ic == SICN - 1))
            rd_sb = ssb.tile([P, TC], F32R, tag="rd_sb")
            b.copy(rd_sb[:], rd_ps[:])
            dT_ps = sp1.tile([P, TC], F32, tag="rd")
            nc.tensor.matmul(dT_ps[:], sUT[:, 2 * P:3 * P], rd_sb[:])
            b.copy(dT_all[:, E * TC:(E + 1) * TC], dT_ps[:])

        # ================= Phase 2: routed experts =================
        with ExitStack() as ectx:
            est = ectx.enter_context(tc.tile_pool(name="ex_st", bufs=2))
            eps = ectx.enter_context(tc.tile_pool(name="ex_ps", bufs=2,
                                                  space="PSUM"))
            ep1 = ectx.enter_context(tc.tile_pool(name="ex_ps1", bufs=1,
                                                  space="PSUM"))

            for e in range(E):
                u_st = est.tile([P, 3 * P], F32, tag="u_st")
                for j, name in enumerate(("Ug", "Uu", "Ud")):
                    nc.sync.dma_start(u_st[:, j * P:(j + 1) * P], ap[name][e])
                uT = est.tile([P, 3 * P], F32R, tag="uT")
                b.transpose_pack(eps, uT,
                                 [u_st[:, j * P:(j + 1) * P] for j in range(3)],
                                 identity)

                cg_st = est.tile([P, I], F32, tag="cg_st")
                nc.sync.dma_start(
                    cg_st[:].rearrange("p (n r) -> p n r", r=P),
                    ap["Cg"][e].rearrange("(n p) r -> p n r", p=P))
                cgT = est.tile([P, I], F32R, tag="cgT")
                b.transpose_pack(eps, cgT,
                                 [cg_st[:, ic * P:(ic + 1) * P]
                                  for ic in range(ICN)], identity)
                cu_st = est.tile([P, I], F32, tag="cu_st")
                nc.sync.dma_start(
                    cu_st[:].rearrange("p (n r) -> p n r", r=P),
                    ap["Cu"][e].rearrange("(n p) r -> p n r", p=P))
                cuT = est.tile([P, I], F32R, tag="cuT")
                b.transpose_pack(eps, cuT,
                                 [cu_st[:, ic * P:(ic + 1) * P]
                                  for ic in range(ICN)], identity)

                gp_ps = ep1.tile([P, TC], F32, tag="gp")
                nc.tensor.matmul(gp_ps[:], uT[:, 0:P], rgT[:])
                g_pre = est.tile([P, TC], F32R, tag="g_pre")
                nc.scalar.copy(g_pre[:], gp_ps[:])
                # broadcast this expert's combine-weight row to all
                # partitions via a K=1 matmul with a ones column
                wrow = est.tile([1, TC], F32, tag="wrow")
                nc.sync.dma_start(wrow[:], we_dram[e:e + 1, :])
                wb_ps = ep1.tile([P, TC], F32, tag="rd", name="wb_ps")
                nc.tensor.matmul(wb_ps[:], ones1[:], wrow[:])
                wb_sb = est.tile([P, TC], F32, tag="wb_sb")
                nc.scalar.copy(wb_sb[:], wb_ps[:])
                up_ps = ep1.tile([P, TC], F32, tag="gp")
                nc.tensor.matmul(up_ps[:], uT[:, P:2 * P], ruT[:])
                # fold the routed combine weight into the u path
                u_pre = est.tile([P, TC], F32R, tag="u_pre")
                nc.vector.tensor_tensor(
                    u_pre[:], up_ps[:], wb_sb[:], ALU.mult)

                rd_ps = ep1.tile([P, TC], F32, tag="rd")
                for ic in range(ICN):
                    g_ps = eps.tile([P, TC], F32, tag="g")
                    nc.tensor.matmul(g_ps[:], cgT[:, ic * P:(ic + 1) * P],
                                     g_pre[:])
                    u_ps = eps.tile([P, TC], F32, tag="u")
                    nc.tensor.matmul(u_ps[:], cuT[:, ic * P:(ic + 1) * P],
                                     u_pre[:])
                    g_sil = est.tile([P, TC], F32, tag="g_sil")
                    nc.scalar.activation(g_sil[:], g_ps[:], AF.Silu)
                    m = est.tile([P, TC], F32R, tag="m")
                    nc.vector.tensor_tensor(m[:], g_sil[:], u_ps[:], ALU.mult)
                    nc.tensor.matmul(rd_ps[:], RdT[:, ic * P:(ic + 1) * P],
                                     m[:], start=(ic == 0),
                                     stop=(ic == ICN - 1))
                rd_sb = est.tile([P, TC], F32R, tag="rd_sb")
                b.copy(rd_sb[:], rd_ps[:])
                dT_ps = ep1.tile([P, TC], F32, tag="rd")
                nc.tensor.matmul(dT_ps[:], uT[:, 2 * P:3 * P], rd_sb[:])
                b.copy(dT_all[:, e * TC:(e + 1) * TC], dT_ps[:])

        # ================= Phase 3: down-projection =================
        with ExitStack() as fctx:
            fsb = fctx.enter_context(tc.tile_pool(name="fi_sb", bufs=1))
            fst = fctx.enter_context(tc.tile_pool(name="fi_st", bufs=4))
            fps = fctx.enter_context(tc.tile_pool(name="fi_ps", bufs=2,
                                                  space="PSUM"))

            y_sb = [fsb.tile([P, H], F32, tag=f"y_sb{t}", name=f"y_sb{t}")
                    for t in range(TCN)]
            for hc in range(HCN):
                y_ps = fps.tile([P, TC], F32, tag="y")
                for j in range(E + 1):
                    if j < E:
                        cd_st = fst.tile([P, P], F32, tag="cd_st")
                        nc.sync.dma_start(
                            cd_st[:], ap["Cd"][j, hc * P:(hc + 1) * P, :])
                        cdT = fst.tile([P, P], F32R, tag="cdT")
                        tp = fps.tile([P, P], F32, tag="tp")
                        nc.tensor.transpose(tp[:], cd_st[:], identity[:])
                        b.copy(cdT[:], tp[:])
                        lhs = cdT[:]
                    else:
                        lhs = s_CdT[:, hc * P:(hc + 1) * P]
                    nc.tensor.matmul(
                        y_ps[:], lhs, dT_all[:, j * TC:(j + 1) * TC],
                        start=(j == 0), stop=(j == E))
                y_hc = fst.tile([P, TC], F32, tag="y_hc")
                b.copy(y_hc[:], y_ps[:])
                for t in range(TCN):
                    yt = fps.tile([P, P], F32, tag="ytp")
                    nc.tensor.transpose(yt[:], y_hc[:, t * P:(t + 1) * P],
                                        identity[:])
                    b.copy(y_sb[t][:, hc * P:(hc + 1) * P], yt[:])
            for t in range(TCN):
                nc.sync.dma_start(y_dram[t * P:(t + 1) * P, :], y_sb[t][:])

    return nc


def kernel(**inputs):
    inputs = {k: np.ascontiguousarray(np.asarray(v, np.float32))
              for k, v in inputs.items()}
    x = inputs["x"].reshape(TFULL, H)

    nc = bacc.Bacc("TRN2", target_bir_lowering=False, debug=False,
                   num_devices=NCORES)
    build(nc)
    nc.finalize()

    in_maps = []
    for c in range(NCORES):
        m = {"x": x[c * TC:(c + 1) * TC]}
        for k, v in inputs.items():
            if k != "x":
                m[k] = v
        in_maps.append(m)

    trace = os.environ.get("KERNEL_TRACE", "0") == "1"
    if trace:
        _ensure_ntff_hook()
    out = run_bass_kernel_spmd(nc, in_maps, list(range(NCORES)), trace=trace)
    global LAST_EXEC_NS, LAST_RESULT
    LAST_EXEC_NS = out.exec_time_ns
    LAST_RESULT = out
    results = out.results
    y = np.concatenate([results[c]["y"] for c in range(NCORES)], axis=0)
    return y.reshape(2, TFULL // 2, H)


LAST_EXEC_NS = None


def _ensure_ntff_hook():
    """Install the axon NTFF profiling hook that the agent image's antenv
    lacks, and keep profile artifacts local (no bucket upload)."""
    import sys
    import types
    import concourse.bass_utils as bu

    bu.upload_artifacts = lambda d: f"local://{d}"
    try:
        from antenv.axon_hooks import get_axon_ntff_profile_hook  # noqa
        return
    except ImportError:
        pass
    import antenv

    mod = types.ModuleType("antenv.axon_hooks")
    _holder = {}
    mod.set_axon_ntff_profile_hook = lambda h: _holder.__setitem__("h", h)
    mod.get_axon_ntff_profile_hook = lambda: _holder.get("h")
    sys.modules["antenv.axon_hooks"] = mod
    antenv.axon_hooks = mod
    if "/root/.axon_site" not in sys.path:
        sys.path.insert(0, "/root/.axon_site")
    from trn_agent_boot.trn_boot import _ntff_profile_via_ctypes

    hook = _ntff_profile_via_ctypes("/opt/axon/libaxon_pjrt.so")
    if hook is not None:
        mod.set_axon_ntff_profile_hook(hook)


# revision 24
# speedup vs baseline: 1.3516x; 1.3516x over previous
"""CUR-DeepSeek-MoE Trainium2 kernel.

Strategy: token-parallel over 8 NeuronCores. Each core processes 512 of the
4096 tokens and reads all weights (replicated). The routed-expert sum is
computed dense-masked (every expert processes the core's 512 tokens, scaled by
the top-4 combine weight, which is zero for non-routed tokens) — numerically
identical to gather/scatter routing. Matmuls run as float32r (full PE rate at
free-dim >= 256, ~tf32 precision). No collectives.

Layout convention: activations live as [feature(part), token(free)] so every
GEMM contracts over partitions; nn.Linear weights [out,in] are transposed on
the PE (128x128 blocks via identity matmul) into [in(part), out(free)].
The final down-projection runs h-chunk-major producing y as [h, t], which is
transposed back to [t, h] before the DMA out.
"""

import os
import numpy as np
from contextlib import ExitStack

import concourse.bass as bass
import concourse.mybir as mybir
import concourse.tile as tile
from concourse import bacc
from concourse.bass_utils import run_bass_kernel_spmd
from concourse.masks import make_identity

F32 = mybir.dt.float32
F32R = mybir.dt.float32r
AF = mybir.ActivationFunctionType
ALU = mybir.AluOpType
AX = mybir.AxisListType

H = 2048
I = 1408
E = 32
RG = 128
SI = 2816
NCORES = 8
TFULL = 4096
TC = TFULL // NCORES          # 512 tokens per core
P = 128
HCN = H // P                  # 16
ICN = I // P                  # 11
SICN = SI // P                # 22
TCN = TC // P                 # 4


def _r(ap):
    return ap.bitcast(F32R)


class _B:
    """Emission helpers bound to one TileContext."""

    def __init__(self, nc):
        self.nc = nc
        self._ctr = 0

    def copy(self, out, in_):
        """PSUM->SBUF copy alternating scalar/vector engines."""
        self._ctr += 1
        if self._ctr % 2 == 0:
            self.nc.scalar.copy(out, in_)
        else:
            self.nc.vector.tensor_copy(out, in_)

    def transpose_pack(self, ps_pool, dst, srcs, identity, tag="tp"):
        """PE-transpose [p,f] SBUF blocks; pack outputs ([f,p]) along dst's
        free dim. Groups <=512 output floats per PSUM bank, one copy/bank."""
        nc = self.nc
        off = 0
        k = 0
        n = len(srcs)
        while k < n:
            width = 0
            take = 0
            while k + take < n:
                w = srcs[k + take].shape[0]
                if width + w > 512:
                    break
                width += w
                take += 1
            outp = srcs[k].shape[1]
            ps = ps_pool.tile([P, 512], F32, tag=tag)
            w0 = 0
            for s in srcs[k:k + take]:
                pw, fp = s.shape[0], s.shape[1]
                nc.tensor.transpose(ps[:fp, w0:w0 + pw], s, identity[:pw, :pw])
                w0 += pw
            self.copy(dst[:outp, off:off + width], ps[:outp, :width])
            off += width
            k += take


def build(nc):
    ap = {}
    specs = {
        "x": [TC, H], "gate_w": [E, H],
        "Rg": [RG, H], "Ru": [RG, H], "Rd": [RG, I],
        "Ug": [E, RG, RG], "Cg": [E, I, RG], "Uu": [E, RG, RG],
        "Cu": [E, I, RG], "Ud": [E, RG, RG], "Cd": [E, H, RG],
        "s_Rg": [RG, H], "s_Ug": [RG, RG], "s_Cg": [SI, RG],
        "s_Ru": [RG, H], "s_Uu": [RG, RG], "s_Cu": [SI, RG],
        "s_Rd": [RG, SI], "s_Ud": [RG, RG], "s_Cd": [H, RG],
    }
    for name, shape in specs.items():
        ap[name] = nc.dram_tensor(name, shape, F32, kind="ExternalInput").ap()
    y_dram = nc.dram_tensor("y", [TC, H], F32, kind="ExternalOutput").ap()
    we_dram = nc.dram_tensor("we_scratch", [E, TC], F32).ap()
    cdT_dram = nc.dram_tensor("cdT_scratch", [E, HCN, P, P], F32R).ap()

    b = _B(nc)
    with tile.TileContext(nc) as tc, ExitStack() as ctx:
        res = ctx.enter_context(tc.tile_pool(name="res", bufs=1))

        identity = res.tile([P, P], F32, tag="ident")
        make_identity(nc, identity[:])

        # Resident across phases:
        rgT = res.tile([P, TC], F32R, tag="rgT")      # [rg, t]
        ruT = res.tile([P, TC], F32R, tag="ruT")
        srgT = res.tile([P, TC], F32R, tag="srgT")
        sruT = res.tile([P, TC], F32R, tag="sruT")
        weT = res.tile([E, TC], F32, tag="weT")      # [e, t] combine weights
        RdT = res.tile([P, I], F32R, tag="RdT")       # [i, rd]
        dT_all = res.tile([P, (E + 1) * TC], F32R, tag="dT_all")  # [rd', e*t]
        ones1 = res.tile([1, P], F32, tag="ones1")
        nc.gpsimd.memset(ones1[:], 1.0)

        # ================= Phase 0: prologue =================
        with ExitStack() as pctx:
            pro = pctx.enter_context(tc.tile_pool(name="pro", bufs=1))
            pst = pctx.enter_context(tc.tile_pool(name="pro_st", bufs=2))
            pps = pctx.enter_context(tc.tile_pool(name="pro_ps", bufs=2,
                                                  space="PSUM"))
            pp1 = pctx.enter_context(tc.tile_pool(name="pro_ps1", bufs=1,
                                                  space="PSUM"))

            # x shard -> xT, token-tile-major: [h(part), t*H + hc*P + hh]
            xT = pro.tile([P, TCN * H], F32R, tag="xT")
            for t in range(TCN):
                xs = pst.tile([P, H], F32, tag="xs")
                nc.sync.dma_start(xs[:], ap["x"][t * P:(t + 1) * P, :])
                b.transpose_pack(
                    pps, xT[:, t * H:(t + 1) * H],
                    [xs[:, hc * P:(hc + 1) * P] for hc in range(HCN)],
                    identity)
            xT_r = xT[:].rearrange("p (t h) -> p t h", h=H)

            def xT_hc(hc):
                # [h128(part), (t, 128 tokens)] strided rhs, N = TC
                return xT_r[:, :, hc * P:(hc + 1) * P]

            # gate_w -> gate_T blocks [h, e] per hc
            gate_nat = pst.tile([E, H], F32, tag="gate_nat")
            nc.sync.dma_start(gate_nat[:], ap["gate_w"][:])
            gate_T = pro.tile([P, HCN * E], F32R, tag="gate_T")
            b.transpose_pack(
                pps, gate_T,
                [gate_nat[:, hc * P:(hc + 1) * P] for hc in range(HCN)],
                identity)

            # Rd -> RdT [i, rd]
            rd_nat = pst.tile([P, H], F32, tag="r_nat")
            nc.sync.dma_start(rd_nat[:, :I], ap["Rd"][:])
            b.transpose_pack(
                pps, RdT,
                [rd_nat[:, ic * P:(ic + 1) * P] for ic in range(ICN)],
                identity)

            # R projections, streamed: transpose then accumulate rg et al.
            for name, dstT in (("Rg", rgT), ("Ru", ruT),
                               ("s_Rg", srgT), ("s_Ru", sruT)):
                nat = pst.tile([P, H], F32, tag="r_nat")
                nc.sync.dma_start(nat[:], ap[name][:])
                rt = pst.tile([P, H], F32R, tag="rT")
                b.transpose_pack(
                    pps, rt,
                    [nat[:, hc * P:(hc + 1) * P] for hc in range(HCN)],
                    identity)
                acc = pp1.tile([P, TC], F32, tag="acc")
                for hc in range(HCN):
                    nc.tensor.matmul(
                        acc[:], rt[:, hc * P:(hc + 1) * P],
                        xT_hc(hc),
                        start=(hc == 0), stop=(hc == HCN - 1))
                b.copy(dstT[:], acc[:])

            # gate logits + top-4 combine weights per token tile
            zeros = pro.tile([P, E], F32, tag="zeros")
            nc.gpsimd.memset(zeros[:], 0.0)
            we_sb = []
            for t in range(TCN):
                lg = pp1.tile([P, E], F32, tag="lg")
                for hc in range(HCN):
                    nc.tensor.matmul(
                        lg[:], xT_r[:, t, hc * P:(hc + 1) * P],
                        gate_T[:, hc * E:(hc + 1) * E],
                        start=(hc == 0), stop=(hc == HCN - 1))
                nmax = pro.tile([P, 1], F32, tag="nmax")
                nc.vector.reduce_max(nmax[:], lg[:], AX.X, negate=True)
                p = pro.tile([P, E], F32, tag=f"p{t}")
                nc.scalar.activation(p[:], lg[:], AF.Exp, bias=nmax[:])
                pc = pro.tile([P, E], F32, tag="pc")
                nc.vector.tensor_copy(pc[:], p[:])
                mx = pro.tile([P, 1], F32, tag="mx")
                msk = pro.tile([P, E], F32, tag="msk")
                msk_i = pro.tile([P, E], mybir.dt.uint8, tag="msk_i")
                for _ in range(3):
                    nc.vector.reduce_max(mx[:], pc[:], AX.X)
                    nc.vector.tensor_scalar(msk_i[:], pc[:], mx[:], None,
                                            ALU.is_equal)
                    nc.vector.copy_predicated(pc[:], msk_i[:], zeros[:])
                nc.vector.reduce_max(mx[:], pc[:], AX.X)  # 4th largest
                nc.vector.tensor_scalar(msk[:], p[:], mx[:], None, ALU.is_ge)
                wu = pro.tile([P, E], F32, tag="wu")
                nc.vector.tensor_tensor(wu[:], p[:], msk[:], ALU.mult)
                den = pro.tile([P, 1], F32, tag="den")
                nc.vector.reduce_sum(den[:], wu[:], AX.X)
                rec = pro.tile([P, 1], F32, tag="rec")
                nc.vector.reciprocal(rec[:], den[:])
                we_t = pro.tile([P, E], F32, tag=f"we{t}")
                nc.vector.tensor_scalar(we_t[:], wu[:], rec[:], None, ALU.mult)
                we_sb.append(we_t)
            b.transpose_pack(pps, weT, [w[:] for w in we_sb], identity)
            nc.sync.dma_start(we_dram[:], weT[:])

        # ================= Phase 1: shared experts =================
        with ExitStack() as sctx:
            ssb = sctx.enter_context(tc.tile_pool(name="sh_sb", bufs=1))
            sst = sctx.enter_context(tc.tile_pool(name="sh_st", bufs=2))
            sps = sctx.enter_context(tc.tile_pool(name="sh_ps", bufs=2,
                                                  space="PSUM"))
            sp1 = sctx.enter_context(tc.tile_pool(name="sh_ps1", bufs=1,
                                                  space="PSUM"))

            sut_st = sst.tile([P, 3 * P], F32, tag="su_st")
            for j, name in enumerate(("s_Ug", "s_Uu", "s_Ud")):
                nc.sync.dma_start(sut_st[:, j * P:(j + 1) * P], ap[name][:])
            sUT = ssb.tile([P, 3 * P], F32R, tag="sUT")
            b.transpose_pack(sps, sUT,
                             [sut_st[:, j * P:(j + 1) * P] for j in range(3)],
                             identity)

            s_CdT = res.tile([P, H], F32R, tag="s_CdT")
            wT = {}
            for name, blocks in (("s_Cg", SICN), ("s_Cu", SICN),
                                 ("s_Rd", SICN), ("s_Cd", HCN)):
                st = sst.tile([P, SICN * P], F32, tag="s_wide_st")
                if name == "s_Rd":
                    nc.sync.dma_start(st[:, :SI], ap[name][:])
                else:
                    nc.sync.dma_start(
                        st[:, :blocks * P].rearrange("p (n r) -> p n r", r=P),
                        ap[name].rearrange("(n p) r -> p n r", p=P))
                srcs = [st[:, ic * P:(ic + 1) * P] for ic in range(blocks)]
                if name == "s_Cd":
                    wT[name] = s_CdT
                else:
                    wT[name] = ssb.tile([P, blocks * P], F32R, tag=f"{name}_T",
                                        name=f"{name}_T")
                b.transpose_pack(sps, wT[name], srcs, identity)

            gp_ps = sp1.tile([P, TC], F32, tag="gp")
            nc.tensor.matmul(gp_ps[:], sUT[:, 0:P], srgT[:])
            g_pre = ssb.tile([P, TC], F32R, tag="g_pre")
            nc.scalar.copy(g_pre[:], gp_ps[:])
            up_ps = sp1.tile([P, TC], F32, tag="gp")
            nc.tensor.matmul(up_ps[:], sUT[:, P:2 * P], sruT[:])
            u_pre = ssb.tile([P, TC], F32R, tag="u_pre")
            nc.vector.tensor_copy(u_pre[:], up_ps[:])

            rd_ps = sp1.tile([P, TC], F32, tag="rd")
            for ic in range(SICN):
                g_ps = sps.tile([P, TC], F32, tag="g")
                nc.tensor.matmul(g_ps[:], wT["s_Cg"][:, ic * P:(ic + 1) * P],
                                 g_pre[:])
                u_ps = sps.tile([P, TC], F32, tag="u")
                nc.tensor.matmul(u_ps[:], wT["s_Cu"][:, ic * P:(ic + 1) * P],
                                 u_pre[:])
                g_sil = sst.tile([P, TC], F32, tag="g_sil")
                nc.scalar.activation(g_sil[:], g_ps[:], AF.Silu)
                m = sst.tile([P, TC], F32R, tag="m")
                nc.vector.tensor_tensor(m[:], g_sil[:], u_ps[:], ALU.mult)
                nc.tensor.matmul(rd_ps[:], wT["s_Rd"][:, ic * P:(ic + 1) * P],
                                 m[:], start=(ic == 0), stop=(ic == SICN - 1))
            rd_sb = ssb.tile([P, TC], F32R, tag="rd_sb")
            b.copy(rd_sb[:], rd_ps[:])
            dT_ps = sp1.tile([P, TC], F32, tag="rd")
            nc.tensor.matmul(dT_ps[:], sUT[:, 2 * P:3 * P], rd_sb[:])
            b.copy(dT_all[:, E * TC:(E + 1) * TC], dT_ps[:])

        # ================= Phase 2: routed experts =================
        with ExitStack() as ectx:
            est = ectx.enter_context(tc.tile_pool(name="ex_st", bufs=2))
            eps = ectx.enter_context(tc.tile_pool(name="ex_ps", bufs=2,
                                                  space="PSUM"))
            ep1 = ectx.enter_context(tc.tile_pool(name="ex_ps1", bufs=1,
                                                  space="PSUM"))

            for e in range(E):
                u_st = est.tile([P, 3 * P], F32, tag="u_st")
                for j, name in enumerate(("Ug", "Uu", "Ud")):
                    nc.sync.dma_start(u_st[:, j * P:(j + 1) * P], ap[name][e])
                uT = est.tile([P, 3 * P], F32R, tag="uT")
                b.transpose_pack(eps, uT,
                                 [u_st[:, j * P:(j + 1) * P] for j in range(3)],
                                 identity)

                cg_st = est.tile([P, I], F32, tag="cg_st")
                nc.sync.dma_start(
                    cg_st[:].rearrange("p (n r) -> p n r", r=P),
                    ap["Cg"][e].rearrange("(n p) r -> p n r", p=P))
                cgT = est.tile([P, I], F32R, tag="cgT")
                b.transpose_pack(eps, cgT,
                                 [cg_st[:, ic * P:(ic + 1) * P]
                                  for ic in range(ICN)], identity)
                cu_st = est.tile([P, I], F32, tag="cu_st")
                nc.sync.dma_start(
                    cu_st[:].rearrange("p (n r) -> p n r", r=P),
                    ap["Cu"][e].rearrange("(n p) r -> p n r", p=P))
                cuT = est.tile([P, I], F32R, tag="cuT")
                b.transpose_pack(eps, cuT,
                                 [cu_st[:, ic * P:(ic + 1) * P]
                                  for ic in range(ICN)], identity)

                gp_ps = ep1.tile([P, TC], F32, tag="gp")
                nc.tensor.matmul(gp_ps[:], uT[:, 0:P], rgT[:])
                g_pre = est.tile([P, TC], F32R, tag="g_pre")
                nc.scalar.copy(g_pre[:], gp_ps[:])
                # broadcast this expert's combine-weight row to all
                # partitions via a K=1 matmul with a ones column
                wrow = est.tile([1, TC], F32, tag="wrow")
                nc.sync.dma_start(wrow[:], we_dram[e:e + 1, :])
                wb_ps = ep1.tile([P, TC], F32, tag="rd", name="wb_ps")
                nc.tensor.matmul(wb_ps[:], ones1[:], wrow[:])
                wb_sb = est.tile([P, TC], F32, tag="wb_sb")
                nc.scalar.copy(wb_sb[:], wb_ps[:])
                up_ps = ep1.tile([P, TC], F32, tag="gp")
                nc.tensor.matmul(up_ps[:], uT[:, P:2 * P], ruT[:])
                # fold the routed combine weight into the u path
                u_pre = est.tile([P, TC], F32R, tag="u_pre")
                nc.vector.tensor_tensor(
                    u_pre[:], up_ps[:], wb_sb[:], ALU.mult)

                rd_ps = ep1.tile([P, TC], F32, tag="rd")
                for ic in range(ICN):
                    g_ps = eps.tile([P, TC], F32, tag="g")
                    nc.tensor.matmul(g_ps[:], cgT[:, ic * P:(ic + 1) * P],
                                     g_pre[:])
                    u_ps = eps.tile([P, TC], F32, tag="u")
                    nc.tensor.matmul(u_ps[:], cuT[:, ic * P:(ic + 1) * P],
                                     u_pre[:])
                    g_sil = est.tile([P, TC], F32, tag="g_sil")
                    nc.scalar.activation(g_sil[:], g_ps[:], AF.Silu)
                    m = est.tile([P, TC], F32R, tag="m")
                    nc.vector.tensor_tensor(m[:], g_sil[:], u_ps[:], ALU.mult)
                    nc.tensor.matmul(rd_ps[:], RdT[:, ic * P:(ic + 1) * P],
                                     m[:], start=(ic == 0),
                                     stop=(ic == ICN - 1))
                rd_sb = est.tile([P, TC], F32R, tag="rd_sb")
                b.copy(rd_sb[:], rd_ps[:])
                dT_ps = ep1.tile([P, TC], F32, tag="rd")
                nc.tensor.matmul(dT_ps[:], uT[:, 2 * P:3 * P], rd_sb[:])
                b.copy(dT_all[:, e * TC:(e + 1) * TC], dT_ps[:])

                # Pre-transpose this expert's Cd into DRAM so the final
                # phase is a dense matmul stream (keeps the PE clock warm).
                cd_st = est.tile([P, H], F32, tag="cd_st")
                nc.sync.dma_start(
                    cd_st[:].rearrange("p (n r) -> p n r", r=P),
                    ap["Cd"][e].rearrange("(n p) r -> p n r", p=P))
                cdT_stage = est.tile([P, H], F32R, tag="cdT_stage")
                b.transpose_pack(eps, cdT_stage,
                                 [cd_st[:, hc * P:(hc + 1) * P]
                                  for hc in range(HCN)], identity)
                nc.sync.dma_start(
                    cdT_dram[e].rearrange("n p r -> p n r"),
                    cdT_stage[:].rearrange("p (n r) -> p n r", r=P))

        # ================= Phase 3: down-projection =================
        with ExitStack() as fctx:
            fsb = fctx.enter_context(tc.tile_pool(name="fi_sb", bufs=1))
            fst = fctx.enter_context(tc.tile_pool(name="fi_st", bufs=4))
            fp2 = fctx.enter_context(tc.tile_pool(name="fi_st2", bufs=2))
            fps = fctx.enter_context(tc.tile_pool(name="fi_ps", bufs=2,
                                                  space="PSUM"))

            y_sb = [fsb.tile([P, H], F32, tag=f"y_sb{t}", name=f"y_sb{t}")
                    for t in range(TCN)]
            for hc in range(HCN):
                cdT_all = fp2.tile([P, E * P], F32R, tag="cdT_all")
                nc.sync.dma_start(
                    cdT_all[:].rearrange("p (n r) -> p n r", r=P),
                    cdT_dram[:, hc].rearrange("n p r -> p n r"))
                y_ps = fps.tile([P, TC], F32, tag="y")
                for j in range(E + 1):
                    if j < E:
                        lhs = cdT_all[:, j * P:(j + 1) * P]
                    else:
                        lhs = s_CdT[:, hc * P:(hc + 1) * P]
                    nc.tensor.matmul(
                        y_ps[:], lhs, dT_all[:, j * TC:(j + 1) * TC],
                        start=(j == 0), stop=(j == E))
                y_hc = fst.tile([P, TC], F32, tag="y_hc")
                b.copy(y_hc[:], y_ps[:])
                for t in range(TCN):
                    yt = fps.tile([P, P], F32, tag="ytp")
                    nc.tensor.transpose(yt[:], y_hc[:, t * P:(t + 1) * P],
                                        identity[:])
                    b.copy(y_sb[t][:, hc * P:(hc + 1) * P], yt[:])
            for t in range(TCN):
                nc.sync.dma_start(y_dram[t * P:(t + 1) * P, :], y_sb[t][:])

    return nc


def kernel(**inputs):
    inputs = {k: np.ascontiguousarray(np.asarray(v, np.float32))
              for k, v in inputs.items()}
    x = inputs["x"].reshape(TFULL, H)

    nc = bacc.Bacc("TRN2", target_bir_lowering=False, debug=False,
                   num_devices=NCORES)
    build(nc)
    nc.finalize()

    in_maps = []
    for c in range(NCORES):
        m = {"x": x[c * TC:(c + 1) * TC]}
        for k, v in inputs.items():
            if k != "x":
                m[k] = v
        in_maps.append(m)

    trace = os.environ.get("KERNEL_TRACE", "0") == "1"
    if trace:
        _ensure_ntff_hook()
    out = run_bass_kernel_spmd(nc, in_maps, list(range(NCORES)), trace=trace)
    global LAST_EXEC_NS, LAST_RESULT
    LAST_EXEC_NS = out.exec_time_ns
    LAST_RESULT = out
    results = out.results
    y = np.concatenate([results[c]["y"] for c in range(NCORES)], axis=0)
    return y.reshape(2, TFULL // 2, H)


LAST_EXEC_NS = None


def _ensure_ntff_hook():
    """Install the axon NTFF profiling hook that the agent image's antenv
    lacks, and keep profile artifacts local (no bucket upload)."""
    import sys
    import types
    import concourse.bass_utils as bu

    bu.upload_artifacts = lambda d: f"local://{d}"
    try:
        from antenv.axon_hooks import get_axon_ntff_profile_hook  # noqa
        return
    except ImportError:
        pass
    import antenv

    mod = types.ModuleType("antenv.axon_hooks")
    _holder = {}
    mod.set_axon_ntff_profile_hook = lambda h: _holder.__setitem__("h", h)
    mod.get_axon_ntff_profile_hook = lambda: _holder.get("h")
    sys.modules["antenv.axon_hooks"] = mod
    antenv.axon_hooks = mod
    if "/root/.axon_site" not in sys.path:
        sys.path.insert(0, "/root/.axon_site")
    from trn_agent_boot.trn_boot import _ntff_profile_via_ctypes

    hook = _ntff_profile_via_ctypes("/opt/axon/libaxon_pjrt.so")
    if hook is not None:
        mod.set_axon_ntff_profile_hook(hook)


# revision 25
# speedup vs baseline: 1.5431x; 1.1416x over previous
"""CUR-DeepSeek-MoE Trainium2 kernel.

Strategy: token-parallel over 8 NeuronCores. Each core processes 512 of the
4096 tokens and reads all weights (replicated). The routed-expert sum is
computed dense-masked (every expert processes the core's 512 tokens, scaled by
the top-4 combine weight, which is zero for non-routed tokens) — numerically
identical to gather/scatter routing. Matmuls run as float32r (full PE rate at
free-dim >= 256, ~tf32 precision). No collectives.

Layout convention: activations live as [feature(part), token(free)] so every
GEMM contracts over partitions; nn.Linear weights [out,in] are transposed on
the PE (128x128 blocks via identity matmul) into [in(part), out(free)].
The final down-projection runs h-chunk-major producing y as [h, t], which is
transposed back to [t, h] before the DMA out.
"""

import os
import numpy as np
from contextlib import ExitStack

import concourse.bass as bass
import concourse.mybir as mybir
import concourse.tile as tile
from concourse import bacc
from concourse.bass_utils import run_bass_kernel_spmd
from concourse.masks import make_identity

F32 = mybir.dt.float32
F32R = mybir.dt.float32r
AF = mybir.ActivationFunctionType
ALU = mybir.AluOpType
AX = mybir.AxisListType

H = 2048
I = 1408
E = 32
RG = 128
SI = 2816
NCORES = 8
TFULL = 4096
TC = TFULL // NCORES          # 512 tokens per core
P = 128
HCN = H // P                  # 16
ICN = I // P                  # 11
SICN = SI // P                # 22
TCN = TC // P                 # 4


def _r(ap):
    return ap.bitcast(F32R)


class _B:
    """Emission helpers bound to one TileContext."""

    def __init__(self, nc):
        self.nc = nc
        self._ctr = 0

    def copy(self, out, in_):
        """PSUM->SBUF copy, biased 2:1 toward the scalar engine (the vector
        engine is the busier of the two in the expert phase)."""
        self._ctr += 1
        if self._ctr % 3 != 0:
            self.nc.scalar.copy(out, in_)
        else:
            self.nc.vector.tensor_copy(out, in_)

    def transpose_pack(self, ps_pool, dst, srcs, identity, tag="tp"):
        """PE-transpose [p,f] SBUF blocks; pack outputs ([f,p]) along dst's
        free dim. Groups <=512 output floats per PSUM bank, one copy/bank."""
        nc = self.nc
        off = 0
        k = 0
        n = len(srcs)
        while k < n:
            width = 0
            take = 0
            while k + take < n:
                w = srcs[k + take].shape[0]
                if width + w > 512:
                    break
                width += w
                take += 1
            outp = srcs[k].shape[1]
            ps = ps_pool.tile([P, 512], F32, tag=tag)
            w0 = 0
            for s in srcs[k:k + take]:
                pw, fp = s.shape[0], s.shape[1]
                nc.tensor.transpose(ps[:fp, w0:w0 + pw], s, identity[:pw, :pw])
                w0 += pw
            self.copy(dst[:outp, off:off + width], ps[:outp, :width])
            off += width
            k += take


def build(nc):
    ap = {}
    specs = {
        "x": [TC, H], "gate_w": [E, H],
        "Rg": [RG, H], "Ru": [RG, H], "Rd": [RG, I],
        "Ug": [E, RG, RG], "Cg": [E, I, RG], "Uu": [E, RG, RG],
        "Cu": [E, I, RG], "Ud": [E, RG, RG], "Cd": [E, H, RG],
        "s_Rg": [RG, H], "s_Ug": [RG, RG], "s_Cg": [SI, RG],
        "s_Ru": [RG, H], "s_Uu": [RG, RG], "s_Cu": [SI, RG],
        "s_Rd": [RG, SI], "s_Ud": [RG, RG], "s_Cd": [H, RG],
    }
    for name, shape in specs.items():
        ap[name] = nc.dram_tensor(name, shape, F32, kind="ExternalInput").ap()
    y_dram = nc.dram_tensor("y", [TC, H], F32, kind="ExternalOutput").ap()
    we_dram = nc.dram_tensor("we_scratch", [E, TC], F32).ap()
    cdT_dram = nc.dram_tensor("cdT_scratch", [E, HCN, P, P], F32R).ap()

    b = _B(nc)
    with tile.TileContext(nc) as tc, ExitStack() as ctx:
        res = ctx.enter_context(tc.tile_pool(name="res", bufs=1))

        identity = res.tile([P, P], F32, tag="ident")
        make_identity(nc, identity[:])

        # Resident across phases:
        rgT = res.tile([P, TC], F32R, tag="rgT")      # [rg, t]
        ruT = res.tile([P, TC], F32R, tag="ruT")
        srgT = res.tile([P, TC], F32R, tag="srgT")
        sruT = res.tile([P, TC], F32R, tag="sruT")
        weT = res.tile([E, TC], F32, tag="weT")      # [e, t] combine weights
        RdT = res.tile([P, I], F32R, tag="RdT")       # [i, rd]
        dT_all = res.tile([P, (E + 1) * TC], F32R, tag="dT_all")  # [rd', e*t]
        ones1 = res.tile([1, P], F32, tag="ones1")
        nc.gpsimd.memset(ones1[:], 1.0)

        # ================= Phase 0: prologue =================
        with ExitStack() as pctx:
            pro = pctx.enter_context(tc.tile_pool(name="pro", bufs=1))
            pst = pctx.enter_context(tc.tile_pool(name="pro_st", bufs=2))
            pps = pctx.enter_context(tc.tile_pool(name="pro_ps", bufs=2,
                                                  space="PSUM"))
            pp1 = pctx.enter_context(tc.tile_pool(name="pro_ps1", bufs=1,
                                                  space="PSUM"))

            # x shard -> xT, token-tile-major: [h(part), t*H + hc*P + hh]
            xT = pro.tile([P, TCN * H], F32R, tag="xT")
            for t in range(TCN):
                xs = pst.tile([P, H], F32, tag="xs")
                nc.sync.dma_start(xs[:], ap["x"][t * P:(t + 1) * P, :])
                b.transpose_pack(
                    pps, xT[:, t * H:(t + 1) * H],
                    [xs[:, hc * P:(hc + 1) * P] for hc in range(HCN)],
                    identity)
            xT_r = xT[:].rearrange("p (t h) -> p t h", h=H)

            def xT_hc(hc):
                # [h128(part), (t, 128 tokens)] strided rhs, N = TC
                return xT_r[:, :, hc * P:(hc + 1) * P]

            # gate_w -> gate_T blocks [h, e] per hc
            gate_nat = pst.tile([E, H], F32, tag="gate_nat")
            nc.sync.dma_start(gate_nat[:], ap["gate_w"][:])
            gate_T = pro.tile([P, HCN * E], F32R, tag="gate_T")
            b.transpose_pack(
                pps, gate_T,
                [gate_nat[:, hc * P:(hc + 1) * P] for hc in range(HCN)],
                identity)

            # Rd -> RdT [i, rd]
            rd_nat = pst.tile([P, H], F32, tag="r_nat")
            nc.sync.dma_start(rd_nat[:, :I], ap["Rd"][:])
            b.transpose_pack(
                pps, RdT,
                [rd_nat[:, ic * P:(ic + 1) * P] for ic in range(ICN)],
                identity)

            # R projections, streamed: transpose then accumulate rg et al.
            for name, dstT in (("Rg", rgT), ("Ru", ruT),
                               ("s_Rg", srgT), ("s_Ru", sruT)):
                nat = pst.tile([P, H], F32, tag="r_nat")
                nc.sync.dma_start(nat[:], ap[name][:])
                rt = pst.tile([P, H], F32R, tag="rT")
                b.transpose_pack(
                    pps, rt,
                    [nat[:, hc * P:(hc + 1) * P] for hc in range(HCN)],
                    identity)
                acc = pp1.tile([P, TC], F32, tag="acc")
                for hc in range(HCN):
                    nc.tensor.matmul(
                        acc[:], rt[:, hc * P:(hc + 1) * P],
                        xT_hc(hc),
                        start=(hc == 0), stop=(hc == HCN - 1))
                b.copy(dstT[:], acc[:])

            # gate logits + top-4 combine weights per token tile
            zeros = pro.tile([P, E], F32, tag="zeros")
            nc.gpsimd.memset(zeros[:], 0.0)
            we_sb = []
            for t in range(TCN):
                lg = pp1.tile([P, E], F32, tag="lg")
                for hc in range(HCN):
                    nc.tensor.matmul(
                        lg[:], xT_r[:, t, hc * P:(hc + 1) * P],
                        gate_T[:, hc * E:(hc + 1) * E],
                        start=(hc == 0), stop=(hc == HCN - 1))
                nmax = pro.tile([P, 1], F32, tag="nmax")
                nc.vector.reduce_max(nmax[:], lg[:], AX.X, negate=True)
                p = pro.tile([P, E], F32, tag=f"p{t}")
                nc.scalar.activation(p[:], lg[:], AF.Exp, bias=nmax[:])
                pc = pro.tile([P, E], F32, tag="pc")
                nc.vector.tensor_copy(pc[:], p[:])
                mx = pro.tile([P, 1], F32, tag="mx")
                msk = pro.tile([P, E], F32, tag="msk")
                msk_i = pro.tile([P, E], mybir.dt.uint8, tag="msk_i")
                for _ in range(3):
                    nc.vector.reduce_max(mx[:], pc[:], AX.X)
                    nc.vector.tensor_scalar(msk_i[:], pc[:], mx[:], None,
                                            ALU.is_equal)
                    nc.vector.copy_predicated(pc[:], msk_i[:], zeros[:])
                nc.vector.reduce_max(mx[:], pc[:], AX.X)  # 4th largest
                nc.vector.tensor_scalar(msk[:], p[:], mx[:], None, ALU.is_ge)
                wu = pro.tile([P, E], F32, tag="wu")
                nc.vector.tensor_tensor(wu[:], p[:], msk[:], ALU.mult)
                den = pro.tile([P, 1], F32, tag="den")
                nc.vector.reduce_sum(den[:], wu[:], AX.X)
                rec = pro.tile([P, 1], F32, tag="rec")
                nc.vector.reciprocal(rec[:], den[:])
                we_t = pro.tile([P, E], F32, tag=f"we{t}")
                nc.vector.tensor_scalar(we_t[:], wu[:], rec[:], None, ALU.mult)
                we_sb.append(we_t)
            b.transpose_pack(pps, weT, [w[:] for w in we_sb], identity)
            nc.sync.dma_start(we_dram[:], weT[:])

        # ================= Phase 1: shared experts =================
        with ExitStack() as sctx:
            ssb = sctx.enter_context(tc.tile_pool(name="sh_sb", bufs=1))
            sst = sctx.enter_context(tc.tile_pool(name="sh_st", bufs=2))
            sps = sctx.enter_context(tc.tile_pool(name="sh_ps", bufs=2,
                                                  space="PSUM"))
            sp1 = sctx.enter_context(tc.tile_pool(name="sh_ps1", bufs=1,
                                                  space="PSUM"))

            sut_st = sst.tile([P, 3 * P], F32, tag="su_st")
            for j, name in enumerate(("s_Ug", "s_Uu", "s_Ud")):
                nc.sync.dma_start(sut_st[:, j * P:(j + 1) * P], ap[name][:])
            sUT = ssb.tile([P, 3 * P], F32R, tag="sUT")
            b.transpose_pack(sps, sUT,
                             [sut_st[:, j * P:(j + 1) * P] for j in range(3)],
                             identity)

            s_CdT = res.tile([P, H], F32R, tag="s_CdT")
            wT = {}
            for name, blocks in (("s_Cg", SICN), ("s_Cu", SICN),
                                 ("s_Rd", SICN), ("s_Cd", HCN)):
                st = sst.tile([P, SICN * P], F32, tag="s_wide_st")
                if name == "s_Rd":
                    nc.sync.dma_start(st[:, :SI], ap[name][:])
                else:
                    nc.sync.dma_start(
                        st[:, :blocks * P].rearrange("p (n r) -> p n r", r=P),
                        ap[name].rearrange("(n p) r -> p n r", p=P))
                srcs = [st[:, ic * P:(ic + 1) * P] for ic in range(blocks)]
                if name == "s_Cd":
                    wT[name] = s_CdT
                else:
                    wT[name] = ssb.tile([P, blocks * P], F32R, tag=f"{name}_T",
                                        name=f"{name}_T")
                b.transpose_pack(sps, wT[name], srcs, identity)

            gp_ps = sp1.tile([P, TC], F32, tag="gp")
            nc.tensor.matmul(gp_ps[:], sUT[:, 0:P], srgT[:])
            g_pre = ssb.tile([P, TC], F32R, tag="g_pre")
            nc.scalar.copy(g_pre[:], gp_ps[:])
            up_ps = sp1.tile([P, TC], F32, tag="gp")
            nc.tensor.matmul(up_ps[:], sUT[:, P:2 * P], sruT[:])
            u_pre = ssb.tile([P, TC], F32R, tag="u_pre")
            nc.vector.tensor_copy(u_pre[:], up_ps[:])

            rd_ps = sp1.tile([P, TC], F32, tag="rd")
            for ic in range(SICN):
                g_ps = sps.tile([P, TC], F32, tag="g")
                nc.tensor.matmul(g_ps[:], wT["s_Cg"][:, ic * P:(ic + 1) * P],
                                 g_pre[:])
                u_ps = sps.tile([P, TC], F32, tag="u")
                nc.tensor.matmul(u_ps[:], wT["s_Cu"][:, ic * P:(ic + 1) * P],
                                 u_pre[:])
                g_sil = sst.tile([P, TC], F32, tag="g_sil")
                nc.scalar.activation(g_sil[:], g_ps[:], AF.Silu)
                m = sst.tile([P, TC], F32R, tag="m")
                nc.vector.tensor_tensor(m[:], g_sil[:], u_ps[:], ALU.mult)
                nc.tensor.matmul(rd_ps[:], wT["s_Rd"][:, ic * P:(ic + 1) * P],
                                 m[:], start=(ic == 0), stop=(ic == SICN - 1))
            rd_sb = ssb.tile([P, TC], F32R, tag="rd_sb")
            b.copy(rd_sb[:], rd_ps[:])
            dT_ps = sp1.tile([P, TC], F32, tag="rd")
            nc.tensor.matmul(dT_ps[:], sUT[:, 2 * P:3 * P], rd_sb[:])
            b.copy(dT_all[:, E * TC:(E + 1) * TC], dT_ps[:])

        # ================= Phase 2: routed experts =================
        with ExitStack() as ectx:
            est = ectx.enter_context(tc.tile_pool(name="ex_st", bufs=2))
            eps = ectx.enter_context(tc.tile_pool(name="ex_ps", bufs=2,
                                                  space="PSUM"))
            ep1 = ectx.enter_context(tc.tile_pool(name="ex_ps1", bufs=1,
                                                  space="PSUM"))

            for e in range(E):
                u_st = est.tile([P, 3 * P], F32, tag="u_st")
                for j, name in enumerate(("Ug", "Uu", "Ud")):
                    nc.sync.dma_start(u_st[:, j * P:(j + 1) * P], ap[name][e])
                uT = est.tile([P, 3 * P], F32R, tag="uT")
                b.transpose_pack(eps, uT,
                                 [u_st[:, j * P:(j + 1) * P] for j in range(3)],
                                 identity)

                cg_st = est.tile([P, I], F32, tag="cg_st")
                nc.sync.dma_start(
                    cg_st[:].rearrange("p (n r) -> p n r", r=P),
                    ap["Cg"][e].rearrange("(n p) r -> p n r", p=P))
                cgT = est.tile([P, I], F32R, tag="cgT")
                b.transpose_pack(eps, cgT,
                                 [cg_st[:, ic * P:(ic + 1) * P]
                                  for ic in range(ICN)], identity)
                cu_st = est.tile([P, I], F32, tag="cu_st")
                nc.sync.dma_start(
                    cu_st[:].rearrange("p (n r) -> p n r", r=P),
                    ap["Cu"][e].rearrange("(n p) r -> p n r", p=P))
                cuT = est.tile([P, I], F32R, tag="cuT")
                b.transpose_pack(eps, cuT,
                                 [cu_st[:, ic * P:(ic + 1) * P]
                                  for ic in range(ICN)], identity)

                gp_ps = ep1.tile([P, TC], F32, tag="gp")
                nc.tensor.matmul(gp_ps[:], uT[:, 0:P], rgT[:])
                g_pre = est.tile([P, TC], F32R, tag="g_pre")
                nc.scalar.copy(g_pre[:], gp_ps[:])
                # broadcast this expert's combine-weight row to all
                # partitions via a K=1 matmul with a ones column
                wrow = est.tile([1, TC], F32, tag="wrow")
                nc.sync.dma_start(wrow[:], we_dram[e:e + 1, :])
                wb_ps = ep1.tile([P, TC], F32, tag="rd", name="wb_ps")
                nc.tensor.matmul(wb_ps[:], ones1[:], wrow[:])
                wb_sb = est.tile([P, TC], F32, tag="wb_sb")
                nc.scalar.copy(wb_sb[:], wb_ps[:])
                up_ps = ep1.tile([P, TC], F32, tag="gp")
                nc.tensor.matmul(up_ps[:], uT[:, P:2 * P], ruT[:])
                # fold the routed combine weight into the u path
                u_pre = est.tile([P, TC], F32R, tag="u_pre")
                nc.vector.tensor_tensor(
                    u_pre[:], up_ps[:], wb_sb[:], ALU.mult)

                rd_ps = ep1.tile([P, TC], F32, tag="rd")
                for ic in range(ICN):
                    g_ps = eps.tile([P, TC], F32, tag="g")
                    nc.tensor.matmul(g_ps[:], cgT[:, ic * P:(ic + 1) * P],
                                     g_pre[:])
                    u_ps = eps.tile([P, TC], F32, tag="u")
                    nc.tensor.matmul(u_ps[:], cuT[:, ic * P:(ic + 1) * P],
                                     u_pre[:])
                    g_sil = est.tile([P, TC], F32, tag="g_sil")
                    nc.scalar.activation(g_sil[:], g_ps[:], AF.Silu)
                    m = est.tile([P, TC], F32R, tag="m")
                    nc.vector.tensor_tensor(m[:], g_sil[:], u_ps[:], ALU.mult)
                    nc.tensor.matmul(rd_ps[:], RdT[:, ic * P:(ic + 1) * P],
                                     m[:], start=(ic == 0),
                                     stop=(ic == ICN - 1))
                rd_sb = est.tile([P, TC], F32R, tag="rd_sb")
                b.copy(rd_sb[:], rd_ps[:])
                dT_ps = ep1.tile([P, TC], F32, tag="rd")
                nc.tensor.matmul(dT_ps[:], uT[:, 2 * P:3 * P], rd_sb[:])
                b.copy(dT_all[:, e * TC:(e + 1) * TC], dT_ps[:])

                # Pre-transpose this expert's Cd into DRAM so the final
                # phase is a dense matmul stream (keeps the PE clock warm).
                cd_st = est.tile([P, H], F32, tag="cd_st")
                nc.sync.dma_start(
                    cd_st[:].rearrange("p (n r) -> p n r", r=P),
                    ap["Cd"][e].rearrange("(n p) r -> p n r", p=P))
                cdT_stage = est.tile([P, H], F32R, tag="cdT_stage")
                b.transpose_pack(eps, cdT_stage,
                                 [cd_st[:, hc * P:(hc + 1) * P]
                                  for hc in range(HCN)], identity)
                nc.sync.dma_start(
                    cdT_dram[e].rearrange("n p r -> p n r"),
                    cdT_stage[:].rearrange("p (n r) -> p n r", r=P))

        # ================= Phase 3: down-projection =================
        with ExitStack() as fctx:
            fsb = fctx.enter_context(tc.tile_pool(name="fi_sb", bufs=1))
            fst = fctx.enter_context(tc.tile_pool(name="fi_st", bufs=4))
            fp2 = fctx.enter_context(tc.tile_pool(name="fi_st2", bufs=2))
            fps = fctx.enter_context(tc.tile_pool(name="fi_ps", bufs=2,
                                                  space="PSUM"))

            y_sb = [fsb.tile([P, H], F32, tag=f"y_sb{t}", name=f"y_sb{t}")
                    for t in range(TCN)]
            for hc in range(HCN):
                cdT_all = fp2.tile([P, E * P], F32R, tag="cdT_all")
                nc.sync.dma_start(
                    cdT_all[:].rearrange("p (n r) -> p n r", r=P),
                    cdT_dram[:, hc].rearrange("n p r -> p n r"))
                y_ps = fps.tile([P, TC], F32, tag="y")
                for j in range(E + 1):
                    if j < E:
                        lhs = cdT_all[:, j * P:(j + 1) * P]
                    else:
                        lhs = s_CdT[:, hc * P:(hc + 1) * P]
                    nc.tensor.matmul(
                        y_ps[:], lhs, dT_all[:, j * TC:(j + 1) * TC],
                        start=(j == 0), stop=(j == E))
                y_hc = fst.tile([P, TC], F32, tag="y_hc")
                b.copy(y_hc[:], y_ps[:])
                for t in range(TCN):
                    yt = fps.tile([P, P], F32, tag="ytp")
                    nc.tensor.transpose(yt[:], y_hc[:, t * P:(t + 1) * P],
                                        identity[:])
                    b.copy(y_sb[t][:, hc * P:(hc + 1) * P], yt[:])
            for t in range(TCN):
                nc.sync.dma_start(y_dram[t * P:(t + 1) * P, :], y_sb[t][:])

    return nc


def kernel(**inputs):
    inputs = {k: np.ascontiguousarray(np.asarray(v, np.float32))
              for k, v in inputs.items()}
    x = inputs["x"].reshape(TFULL, H)

    nc = bacc.Bacc("TRN2", target_bir_lowering=False, debug=False,
                   num_devices=NCORES)
    build(nc)
    nc.finalize()

    in_maps = []
    for c in range(NCORES):
        m = {"x": x[c * TC:(c + 1) * TC]}
        for k, v in inputs.items():
            if k != "x":
                m[k] = v
        in_maps.append(m)

    trace = os.environ.get("KERNEL_TRACE", "0") == "1"
    if trace:
        _ensure_ntff_hook()
    out = run_bass_kernel_spmd(nc, in_maps, list(range(NCORES)), trace=trace)
    global LAST_EXEC_NS, LAST_RESULT
    LAST_EXEC_NS = out.exec_time_ns
    LAST_RESULT = out
    results = out.results
    y = np.concatenate([results[c]["y"] for c in range(NCORES)], axis=0)
    return y.reshape(2, TFULL // 2, H)


LAST_EXEC_NS = None


def _ensure_ntff_hook():
    """Install the axon NTFF profiling hook that the agent image's antenv
    lacks, and keep profile artifacts local (no bucket upload)."""
    import sys
    import types
    import concourse.bass_utils as bu

    bu.upload_artifacts = lambda d: f"local://{d}"
    try:
        from antenv.axon_hooks import get_axon_ntff_profile_hook  # noqa
        return
    except ImportError:
        pass
    import antenv

    mod = types.ModuleType("antenv.axon_hooks")
    _holder = {}
    mod.set_axon_ntff_profile_hook = lambda h: _holder.__setitem__("h", h)
    mod.get_axon_ntff_profile_hook = lambda: _holder.get("h")
    sys.modules["antenv.axon_hooks"] = mod
    antenv.axon_hooks = mod
    if "/root/.axon_site" not in sys.path:
        sys.path.insert(0, "/root/.axon_site")
    from trn_agent_boot.trn_boot import _ntff_profile_via_ctypes

    hook = _ntff_profile_via_ctypes("/opt/axon/libaxon_pjrt.so")
    if hook is not None:
        mod.set_axon_ntff_profile_hook(hook)


# revision 27
# speedup vs baseline: 1.6315x; 1.0573x over previous
"""CUR-DeepSeek-MoE Trainium2 kernel.

Strategy: token-parallel over 8 NeuronCores. Each core processes 512 of the
4096 tokens and reads all weights (replicated). The routed-expert sum is
computed dense-masked (every expert processes the core's 512 tokens, scaled by
the top-4 combine weight, which is zero for non-routed tokens) — numerically
identical to gather/scatter routing. Matmuls run as float32r (full PE rate at
free-dim >= 256, ~tf32 precision). No collectives.

Layout convention: activations live as [feature(part), token(free)] so every
GEMM contracts over partitions; nn.Linear weights [out,in] are transposed on
the PE (128x128 blocks via identity matmul) into [in(part), out(free)].
The final down-projection runs h-chunk-major producing y as [h, t], which is
transposed back to [t, h] before the DMA out.
"""

import os
import numpy as np
from contextlib import ExitStack

import concourse.bass as bass
import concourse.mybir as mybir
import concourse.tile as tile
from concourse import bacc
from concourse.bass_utils import run_bass_kernel_spmd
from concourse.masks import make_identity

F32 = mybir.dt.float32
F32R = mybir.dt.float32r
AF = mybir.ActivationFunctionType
ALU = mybir.AluOpType
AX = mybir.AxisListType

H = 2048
I = 1408
E = 32
RG = 128
SI = 2816
NCORES = 8
TFULL = 4096
TC = TFULL // NCORES          # 512 tokens per core
P = 128
HCN = H // P                  # 16
ICN = I // P                  # 11
SICN = SI // P                # 22
TCN = TC // P                 # 4


def _r(ap):
    return ap.bitcast(F32R)


class _B:
    """Emission helpers bound to one TileContext."""

    def __init__(self, nc):
        self.nc = nc
        self._ctr = 0

    def copy(self, out, in_):
        """PSUM->SBUF copy, biased 2:1 toward the scalar engine (the vector
        engine is the busier of the two in the expert phase)."""
        self._ctr += 1
        if self._ctr % 4 != 0:
            self.nc.scalar.copy(out, in_)
        else:
            self.nc.vector.tensor_copy(out, in_)

    def transpose_pack(self, ps_pool, dst, srcs, identity, tag="tp"):
        """PE-transpose [p,f] SBUF blocks; pack outputs ([f,p]) along dst's
        free dim. Groups <=512 output floats per PSUM bank, one copy/bank."""
        nc = self.nc
        off = 0
        k = 0
        n = len(srcs)
        while k < n:
            width = 0
            take = 0
            while k + take < n:
                w = srcs[k + take].shape[0]
                if width + w > 512:
                    break
                width += w
                take += 1
            outp = srcs[k].shape[1]
            ps = ps_pool.tile([P, 512], F32, tag=tag)
            w0 = 0
            for s in srcs[k:k + take]:
                pw, fp = s.shape[0], s.shape[1]
                nc.tensor.transpose(ps[:fp, w0:w0 + pw], s, identity[:pw, :pw])
                w0 += pw
            self.copy(dst[:outp, off:off + width], ps[:outp, :width])
            off += width
            k += take


def build(nc):
    ap = {}
    specs = {
        "x": [TC, H], "gate_w": [E, H],
        "Rg": [RG, H], "Ru": [RG, H], "Rd": [RG, I],
        "Ug": [E, RG, RG], "Cg": [E, I, RG], "Uu": [E, RG, RG],
        "Cu": [E, I, RG], "Ud": [E, RG, RG], "Cd": [E, H, RG],
        "s_Rg": [RG, H], "s_Ug": [RG, RG], "s_Cg": [SI, RG],
        "s_Ru": [RG, H], "s_Uu": [RG, RG], "s_Cu": [SI, RG],
        "s_Rd": [RG, SI], "s_Ud": [RG, RG], "s_Cd": [H, RG],
    }
    for name, shape in specs.items():
        ap[name] = nc.dram_tensor(name, shape, F32, kind="ExternalInput").ap()
    y_dram = nc.dram_tensor("y", [TC, H], F32, kind="ExternalOutput").ap()
    we_dram = nc.dram_tensor("we_scratch", [E, TC], F32).ap()
    cdT_dram = nc.dram_tensor("cdT_scratch", [E, HCN, P, P], F32R).ap()

    b = _B(nc)
    with tile.TileContext(nc) as tc, ExitStack() as ctx:
        res = ctx.enter_context(tc.tile_pool(name="res", bufs=1))

        identity = res.tile([P, P], F32, tag="ident")
        make_identity(nc, identity[:])

        # Resident across phases:
        rgT = res.tile([P, TC], F32R, tag="rgT")      # [rg, t]
        ruT = res.tile([P, TC], F32R, tag="ruT")
        srgT = res.tile([P, TC], F32R, tag="srgT")
        sruT = res.tile([P, TC], F32R, tag="sruT")
        weT = res.tile([E, TC], F32, tag="weT")      # [e, t] combine weights
        RdT = res.tile([P, I], F32R, tag="RdT")       # [i, rd]
        dT_all = res.tile([P, (E + 1) * TC], F32R, tag="dT_all")  # [rd', e*t]
        ones1f = res.tile([1, P], F32, tag="ones1f")
        nc.gpsimd.memset(ones1f[:], 1.0)
        ones1 = res.tile([1, P], F32R, tag="ones1")
        nc.scalar.copy(ones1[:], ones1f[:])

        # ================= Phase 0: prologue =================
        with ExitStack() as pctx:
            pro = pctx.enter_context(tc.tile_pool(name="pro", bufs=1))
            pst = pctx.enter_context(tc.tile_pool(name="pro_st", bufs=2))
            pps = pctx.enter_context(tc.tile_pool(name="pro_ps", bufs=2,
                                                  space="PSUM"))
            pp1 = pctx.enter_context(tc.tile_pool(name="pro_ps1", bufs=1,
                                                  space="PSUM"))

            # x shard -> xT, token-tile-major: [h(part), t*H + hc*P + hh]
            xT = pro.tile([P, TCN * H], F32R, tag="xT")
            for t in range(TCN):
                xs = pst.tile([P, H], F32, tag="xs")
                nc.sync.dma_start(xs[:], ap["x"][t * P:(t + 1) * P, :])
                b.transpose_pack(
                    pps, xT[:, t * H:(t + 1) * H],
                    [xs[:, hc * P:(hc + 1) * P] for hc in range(HCN)],
                    identity)
            xT_r = xT[:].rearrange("p (t h) -> p t h", h=H)

            def xT_hc(hc):
                # [h128(part), (t, 128 tokens)] strided rhs, N = TC
                return xT_r[:, :, hc * P:(hc + 1) * P]

            # gate_w -> gate_T blocks [h, e] per hc
            gate_nat = pst.tile([E, H], F32, tag="gate_nat")
            nc.sync.dma_start(gate_nat[:], ap["gate_w"][:])
            gate_T = pro.tile([P, HCN * E], F32R, tag="gate_T")
            b.transpose_pack(
                pps, gate_T,
                [gate_nat[:, hc * P:(hc + 1) * P] for hc in range(HCN)],
                identity)

            # Rd -> RdT [i, rd]
            rd_nat = pst.tile([P, H], F32, tag="r_nat")
            nc.sync.dma_start(rd_nat[:, :I], ap["Rd"][:])
            b.transpose_pack(
                pps, RdT,
                [rd_nat[:, ic * P:(ic + 1) * P] for ic in range(ICN)],
                identity)

            # R projections, streamed: transpose then accumulate rg et al.
            for name, dstT in (("Rg", rgT), ("Ru", ruT),
                               ("s_Rg", srgT), ("s_Ru", sruT)):
                nat = pst.tile([P, H], F32, tag="r_nat")
                nc.sync.dma_start(nat[:], ap[name][:])
                rt = pst.tile([P, H], F32R, tag="rT")
                b.transpose_pack(
                    pps, rt,
                    [nat[:, hc * P:(hc + 1) * P] for hc in range(HCN)],
                    identity)
                acc = pp1.tile([P, TC], F32, tag="acc")
                for hc in range(HCN):
                    nc.tensor.matmul(
                        acc[:], rt[:, hc * P:(hc + 1) * P],
                        xT_hc(hc),
                        start=(hc == 0), stop=(hc == HCN - 1))
                b.copy(dstT[:], acc[:])

            # gate logits + top-4 combine weights per token tile
            zeros = pro.tile([P, E], F32, tag="zeros")
            nc.gpsimd.memset(zeros[:], 0.0)
            we_sb = []
            for t in range(TCN):
                lg = pp1.tile([P, E], F32, tag="lg")
                for hc in range(HCN):
                    nc.tensor.matmul(
                        lg[:], xT_r[:, t, hc * P:(hc + 1) * P],
                        gate_T[:, hc * E:(hc + 1) * E],
                        start=(hc == 0), stop=(hc == HCN - 1))
                nmax = pro.tile([P, 1], F32, tag="nmax")
                nc.vector.reduce_max(nmax[:], lg[:], AX.X, negate=True)
                p = pro.tile([P, E], F32, tag=f"p{t}")
                nc.scalar.activation(p[:], lg[:], AF.Exp, bias=nmax[:])
                pc = pro.tile([P, E], F32, tag="pc")
                nc.vector.tensor_copy(pc[:], p[:])
                mx = pro.tile([P, 1], F32, tag="mx")
                msk = pro.tile([P, E], F32, tag="msk")
                msk_i = pro.tile([P, E], mybir.dt.uint8, tag="msk_i")
                for _ in range(3):
                    nc.vector.reduce_max(mx[:], pc[:], AX.X)
                    nc.vector.tensor_scalar(msk_i[:], pc[:], mx[:], None,
                                            ALU.is_equal)
                    nc.vector.copy_predicated(pc[:], msk_i[:], zeros[:])
                nc.vector.reduce_max(mx[:], pc[:], AX.X)  # 4th largest
                nc.vector.tensor_scalar(msk[:], p[:], mx[:], None, ALU.is_ge)
                wu = pro.tile([P, E], F32, tag="wu")
                nc.vector.tensor_tensor(wu[:], p[:], msk[:], ALU.mult)
                den = pro.tile([P, 1], F32, tag="den")
                nc.vector.reduce_sum(den[:], wu[:], AX.X)
                rec = pro.tile([P, 1], F32, tag="rec")
                nc.vector.reciprocal(rec[:], den[:])
                we_t = pro.tile([P, E], F32, tag=f"we{t}")
                nc.vector.tensor_scalar(we_t[:], wu[:], rec[:], None, ALU.mult)
                we_sb.append(we_t)
            b.transpose_pack(pps, weT, [w[:] for w in we_sb], identity)
            nc.sync.dma_start(we_dram[:], weT[:])

        # ================= Phase 1: shared experts =================
        with ExitStack() as sctx:
            ssb = sctx.enter_context(tc.tile_pool(name="sh_sb", bufs=1))
            sst = sctx.enter_context(tc.tile_pool(name="sh_st", bufs=2))
            sps = sctx.enter_context(tc.tile_pool(name="sh_ps", bufs=2,
                                                  space="PSUM"))
            sp1 = sctx.enter_context(tc.tile_pool(name="sh_ps1", bufs=1,
                                                  space="PSUM"))

            sut_st = sst.tile([P, 3 * P], F32, tag="su_st")
            for j, name in enumerate(("s_Ug", "s_Uu", "s_Ud")):
                nc.sync.dma_start(sut_st[:, j * P:(j + 1) * P], ap[name][:])
            sUT = ssb.tile([P, 3 * P], F32R, tag="sUT")
            b.transpose_pack(sps, sUT,
                             [sut_st[:, j * P:(j + 1) * P] for j in range(3)],
                             identity)

            s_CdT = res.tile([P, H], F32R, tag="s_CdT")
            wT = {}
            for name, blocks in (("s_Cg", SICN), ("s_Cu", SICN),
                                 ("s_Rd", SICN), ("s_Cd", HCN)):
                st = sst.tile([P, SICN * P], F32, tag="s_wide_st")
                if name == "s_Rd":
                    nc.sync.dma_start(st[:, :SI], ap[name][:])
                else:
                    nc.sync.dma_start(
                        st[:, :blocks * P].rearrange("p (n r) -> p n r", r=P),
                        ap[name].rearrange("(n p) r -> p n r", p=P))
                srcs = [st[:, ic * P:(ic + 1) * P] for ic in range(blocks)]
                if name == "s_Cd":
                    wT[name] = s_CdT
                else:
                    wT[name] = ssb.tile([P, blocks * P], F32R, tag=f"{name}_T",
                                        name=f"{name}_T")
                b.transpose_pack(sps, wT[name], srcs, identity)

            gp_ps = sp1.tile([P, TC], F32, tag="gp")
            nc.tensor.matmul(gp_ps[:], sUT[:, 0:P], srgT[:])
            g_pre = ssb.tile([P, TC], F32R, tag="g_pre")
            nc.scalar.copy(g_pre[:], gp_ps[:])
            up_ps = sp1.tile([P, TC], F32, tag="gp")
            nc.tensor.matmul(up_ps[:], sUT[:, P:2 * P], sruT[:])
            u_pre = ssb.tile([P, TC], F32R, tag="u_pre")
            nc.vector.tensor_copy(u_pre[:], up_ps[:])

            rd_ps = sp1.tile([P, TC], F32, tag="rd")
            for ic in range(SICN):
                g_ps = sps.tile([P, TC], F32, tag="g")
                nc.tensor.matmul(g_ps[:], wT["s_Cg"][:, ic * P:(ic + 1) * P],
                                 g_pre[:])
                u_ps = sps.tile([P, TC], F32, tag="u")
                nc.tensor.matmul(u_ps[:], wT["s_Cu"][:, ic * P:(ic + 1) * P],
                                 u_pre[:])
                g_sil = sst.tile([P, TC], F32, tag="g_sil")
                nc.scalar.activation(g_sil[:], g_ps[:], AF.Silu)
                m = sst.tile([P, TC], F32R, tag="m")
                nc.vector.tensor_tensor(m[:], g_sil[:], u_ps[:], ALU.mult)
                nc.tensor.matmul(rd_ps[:], wT["s_Rd"][:, ic * P:(ic + 1) * P],
                                 m[:], start=(ic == 0), stop=(ic == SICN - 1))
            rd_sb = ssb.tile([P, TC], F32R, tag="rd_sb")
            b.copy(rd_sb[:], rd_ps[:])
            dT_ps = sp1.tile([P, TC], F32, tag="rd")
            nc.tensor.matmul(dT_ps[:], sUT[:, 2 * P:3 * P], rd_sb[:])
            b.copy(dT_all[:, E * TC:(E + 1) * TC], dT_ps[:])

        # ================= Phase 2: routed experts =================
        with ExitStack() as ectx:
            est = ectx.enter_context(tc.tile_pool(name="ex_st", bufs=2))
            eps = ectx.enter_context(tc.tile_pool(name="ex_ps", bufs=2,
                                                  space="PSUM"))
            ep1 = ectx.enter_context(tc.tile_pool(name="ex_ps1", bufs=1,
                                                  space="PSUM"))

            for e in range(E):
                u_st = est.tile([P, 3 * P], F32, tag="u_st")
                for j, name in enumerate(("Ug", "Uu", "Ud")):
                    nc.sync.dma_start(u_st[:, j * P:(j + 1) * P], ap[name][e])
                uT = est.tile([P, 3 * P], F32R, tag="uT")
                b.transpose_pack(eps, uT,
                                 [u_st[:, j * P:(j + 1) * P] for j in range(3)],
                                 identity)

                cg_st = est.tile([P, I], F32, tag="cg_st")
                nc.sync.dma_start(
                    cg_st[:].rearrange("p (n r) -> p n r", r=P),
                    ap["Cg"][e].rearrange("(n p) r -> p n r", p=P))
                cgT = est.tile([P, I], F32R, tag="cgT")
                b.transpose_pack(eps, cgT,
                                 [cg_st[:, ic * P:(ic + 1) * P]
                                  for ic in range(ICN)], identity)
                cu_st = est.tile([P, I], F32, tag="cu_st")
                nc.sync.dma_start(
                    cu_st[:].rearrange("p (n r) -> p n r", r=P),
                    ap["Cu"][e].rearrange("(n p) r -> p n r", p=P))
                cuT = est.tile([P, I], F32R, tag="cuT")
                b.transpose_pack(eps, cuT,
                                 [cu_st[:, ic * P:(ic + 1) * P]
                                  for ic in range(ICN)], identity)

                gp_ps = ep1.tile([P, TC], F32, tag="gp")
                nc.tensor.matmul(gp_ps[:], uT[:, 0:P], rgT[:])
                g_pre = est.tile([P, TC], F32R, tag="g_pre")
                nc.scalar.copy(g_pre[:], gp_ps[:])
                # broadcast this expert's combine-weight row to all
                # partitions via a K=1 matmul with a ones column
                wrow = est.tile([1, TC], F32R, tag="wrow")
                nc.gpsimd.dma_start(wrow[:], we_dram[e:e + 1, :])
                wb_ps = ep1.tile([P, TC], F32, tag="rd", name="wb_ps")
                nc.tensor.matmul(wb_ps[:], ones1[:], wrow[:])
                wb_sb = est.tile([P, TC], F32, tag="wb_sb")
                nc.scalar.copy(wb_sb[:], wb_ps[:])
                up_ps = ep1.tile([P, TC], F32, tag="gp")
                nc.tensor.matmul(up_ps[:], uT[:, P:2 * P], ruT[:])
                # fold the routed combine weight into the u path
                u_pre = est.tile([P, TC], F32R, tag="u_pre")
                nc.vector.tensor_tensor(
                    u_pre[:], up_ps[:], wb_sb[:], ALU.mult)

                rd_ps = ep1.tile([P, TC], F32, tag="rd")
                for ic in range(ICN):
                    g_ps = eps.tile([P, TC], F32, tag="g")
                    nc.tensor.matmul(g_ps[:], cgT[:, ic * P:(ic + 1) * P],
                                     g_pre[:])
                    u_ps = eps.tile([P, TC], F32, tag="u")
                    nc.tensor.matmul(u_ps[:], cuT[:, ic * P:(ic + 1) * P],
                                     u_pre[:])
                    g_sil = est.tile([P, TC], F32, tag="g_sil")
                    nc.scalar.activation(g_sil[:], g_ps[:], AF.Silu)
                    m = est.tile([P, TC], F32R, tag="m")
                    nc.vector.tensor_tensor(m[:], g_sil[:], u_ps[:], ALU.mult)
                    nc.tensor.matmul(rd_ps[:], RdT[:, ic * P:(ic + 1) * P],
                                     m[:], start=(ic == 0),
                                     stop=(ic == ICN - 1))
                rd_sb = est.tile([P, TC], F32R, tag="rd_sb")
                b.copy(rd_sb[:], rd_ps[:])
                dT_ps = ep1.tile([P, TC], F32, tag="rd")
                nc.tensor.matmul(dT_ps[:], uT[:, 2 * P:3 * P], rd_sb[:])
                b.copy(dT_all[:, e * TC:(e + 1) * TC], dT_ps[:])

                # Pre-transpose this expert's Cd into DRAM so the final
                # phase is a dense matmul stream (keeps the PE clock warm).
                cd_st = est.tile([P, H], F32, tag="cd_st")
                nc.sync.dma_start(
                    cd_st[:].rearrange("p (n r) -> p n r", r=P),
                    ap["Cd"][e].rearrange("(n p) r -> p n r", p=P))
                cdT_stage = est.tile([P, H], F32R, tag="cdT_stage")
                b.transpose_pack(eps, cdT_stage,
                                 [cd_st[:, hc * P:(hc + 1) * P]
                                  for hc in range(HCN)], identity)
                nc.sync.dma_start(
                    cdT_dram[e].rearrange("n p r -> p n r"),
                    cdT_stage[:].rearrange("p (n r) -> p n r", r=P))

        # ================= Phase 3: down-projection =================
        with ExitStack() as fctx:
            fsb = fctx.enter_context(tc.tile_pool(name="fi_sb", bufs=1))
            fst = fctx.enter_context(tc.tile_pool(name="fi_st", bufs=4))
            fp2 = fctx.enter_context(tc.tile_pool(name="fi_st2", bufs=2))
            fps = fctx.enter_context(tc.tile_pool(name="fi_ps", bufs=2,
                                                  space="PSUM"))

            y_sb = [fsb.tile([P, H], F32, tag=f"y_sb{t}", name=f"y_sb{t}")
                    for t in range(TCN)]
            for hc in range(HCN):
                cdT_all = fp2.tile([P, E * P], F32R, tag="cdT_all")
                nc.sync.dma_start(
                    cdT_all[:].rearrange("p (n r) -> p n r", r=P),
                    cdT_dram[:, hc].rearrange("n p r -> p n r"))
                y_ps = fps.tile([P, TC], F32, tag="y")
                for j in range(E + 1):
                    if j < E:
                        lhs = cdT_all[:, j * P:(j + 1) * P]
                    else:
                        lhs = s_CdT[:, hc * P:(hc + 1) * P]
                    nc.tensor.matmul(
                        y_ps[:], lhs, dT_all[:, j * TC:(j + 1) * TC],
                        start=(j == 0), stop=(j == E))
                y_hc = fst.tile([P, TC], F32, tag="y_hc")
                b.copy(y_hc[:], y_ps[:])
                for t in range(TCN):
                    yt = fps.tile([P, P], F32, tag="ytp")
                    nc.tensor.transpose(yt[:], y_hc[:, t * P:(t + 1) * P],
                                        identity[:])
                    b.copy(y_sb[t][:, hc * P:(hc + 1) * P], yt[:])
            for t in range(TCN):
                nc.sync.dma_start(y_dram[t * P:(t + 1) * P, :], y_sb[t][:])

    return nc


def kernel(**inputs):
    inputs = {k: np.ascontiguousarray(np.asarray(v, np.float32))
              for k, v in inputs.items()}
    x = inputs["x"].reshape(TFULL, H)

    nc = bacc.Bacc("TRN2", target_bir_lowering=False, debug=False,
                   num_devices=NCORES)
    build(nc)
    nc.finalize()

    in_maps = []
    for c in range(NCORES):
        m = {"x": x[c * TC:(c + 1) * TC]}
        for k, v in inputs.items():
            if k != "x":
                m[k] = v
        in_maps.append(m)

    trace = os.environ.get("KERNEL_TRACE", "0") == "1"
    if trace:
        _ensure_ntff_hook()
    out = run_bass_kernel_spmd(nc, in_maps, list(range(NCORES)), trace=trace)
    global LAST_EXEC_NS, LAST_RESULT
    LAST_EXEC_NS = out.exec_time_ns
    LAST_RESULT = out
    results = out.results
    y = np.concatenate([results[c]["y"] for c in range(NCORES)], axis=0)
    return y.reshape(2, TFULL // 2, H)


LAST_EXEC_NS = None


def _ensure_ntff_hook():
    """Install the axon NTFF profiling hook that the agent image's antenv
    lacks, and keep profile artifacts local (no bucket upload)."""
    import sys
    import types
    import concourse.bass_utils as bu

    bu.upload_artifacts = lambda d: f"local://{d}"
    try:
        from antenv.axon_hooks import get_axon_ntff_profile_hook  # noqa
        return
    except ImportError:
        pass
    import antenv

    mod = types.ModuleType("antenv.axon_hooks")
    _holder = {}
    mod.set_axon_ntff_profile_hook = lambda h: _holder.__setitem__("h", h)
    mod.get_axon_ntff_profile_hook = lambda: _holder.get("h")
    sys.modules["antenv.axon_hooks"] = mod
    antenv.axon_hooks = mod
    if "/root/.axon_site" not in sys.path:
        sys.path.insert(0, "/root/.axon_site")
    from trn_agent_boot.trn_boot import _ntff_profile_via_ctypes

    hook = _ntff_profile_via_ctypes("/opt/axon/libaxon_pjrt.so")
    if hook is not None:
        mod.set_axon_ntff_profile_hook(hook)


# revision 30
# speedup vs baseline: 1.7083x; 1.0471x over previous
"""CUR-DeepSeek-MoE Trainium2 kernel.

Strategy: token-parallel over 8 NeuronCores. Each core processes 512 of the
4096 tokens and reads all weights (replicated). The routed-expert sum is
computed dense-masked (every expert processes the core's 512 tokens, scaled by
the top-4 combine weight, which is zero for non-routed tokens) — numerically
identical to gather/scatter routing. Matmuls run as float32r (full PE rate at
free-dim >= 256, ~tf32 precision). No collectives.

Layout convention: activations live as [feature(part), token(free)] so every
GEMM contracts over partitions; nn.Linear weights [out,in] are transposed on
the PE (128x128 blocks via identity matmul) into [in(part), out(free)].
The final down-projection runs h-chunk-major producing y as [h, t], which is
transposed back to [t, h] before the DMA out.
"""

import os
import numpy as np
from contextlib import ExitStack

import concourse.bass as bass
import concourse.mybir as mybir
import concourse.tile as tile
from concourse import bacc
from concourse.bass_utils import run_bass_kernel_spmd
from concourse.masks import make_identity

F32 = mybir.dt.float32
F32R = mybir.dt.float32r
AF = mybir.ActivationFunctionType
ALU = mybir.AluOpType
AX = mybir.AxisListType

H = 2048
I = 1408
E = 32
RG = 128
SI = 2816
NCORES = 8
TFULL = 4096
TC = TFULL // NCORES          # 512 tokens per core
P = 128
HCN = H // P                  # 16
ICN = I // P                  # 11
SICN = SI // P                # 22
TCN = TC // P                 # 4


def _r(ap):
    return ap.bitcast(F32R)


class _B:
    """Emission helpers bound to one TileContext."""

    def __init__(self, nc):
        self.nc = nc
        self._ctr = 0

    def copy(self, out, in_):
        """PSUM->SBUF copy, biased 2:1 toward the scalar engine (the vector
        engine is the busier of the two in the expert phase)."""
        self._ctr += 1
        if self._ctr % 4 != 0:
            self.nc.scalar.copy(out, in_)
        else:
            self.nc.vector.tensor_copy(out, in_)

    def transpose_pack(self, ps_pool, dst, srcs, identity, tag="tp"):
        """PE-transpose [p,f] SBUF blocks; pack outputs ([f,p]) along dst's
        free dim. Groups <=512 output floats per PSUM bank, one copy/bank."""
        nc = self.nc
        off = 0
        k = 0
        n = len(srcs)
        while k < n:
            width = 0
            take = 0
            while k + take < n:
                w = srcs[k + take].shape[0]
                if width + w > 512:
                    break
                width += w
                take += 1
            outp = srcs[k].shape[1]
            ps = ps_pool.tile([P, 512], F32, tag=tag)
            w0 = 0
            for s in srcs[k:k + take]:
                pw, fp = s.shape[0], s.shape[1]
                nc.tensor.transpose(ps[:fp, w0:w0 + pw], s, identity[:pw, :pw])
                w0 += pw
            self.copy(dst[:outp, off:off + width], ps[:outp, :width])
            off += width
            k += take


def build(nc):
    ap = {}
    specs = {
        "x": [TC, H], "gate_w": [E, H],
        "Rg": [RG, H], "Ru": [RG, H], "Rd": [RG, I],
        "Ug": [E, RG, RG], "Cg": [E, I, RG], "Uu": [E, RG, RG],
        "Cu": [E, I, RG], "Ud": [E, RG, RG], "Cd": [E, H, RG],
        "s_Rg": [RG, H], "s_Ug": [RG, RG], "s_Cg": [SI, RG],
        "s_Ru": [RG, H], "s_Uu": [RG, RG], "s_Cu": [SI, RG],
        "s_Rd": [RG, SI], "s_Ud": [RG, RG], "s_Cd": [H, RG],
    }
    for name, shape in specs.items():
        ap[name] = nc.dram_tensor(name, shape, F32, kind="ExternalInput").ap()
    y_dram = nc.dram_tensor("y", [TC, H], F32, kind="ExternalOutput").ap()
    we_dram = nc.dram_tensor("we_scratch", [E, TC], F32).ap()
    cdT_dram = nc.dram_tensor("cdT_scratch", [E, HCN, P, P], F32R).ap()

    b = _B(nc)
    with tile.TileContext(nc) as tc, ExitStack() as ctx:
        res = ctx.enter_context(tc.tile_pool(name="res", bufs=1))

        identity = res.tile([P, P], F32, tag="ident")
        make_identity(nc, identity[:])

        # Resident across phases:
        rgT = res.tile([P, TC], F32R, tag="rgT")      # [rg, t]
        ruT = res.tile([P, TC], F32R, tag="ruT")
        srgT = res.tile([P, TC], F32R, tag="srgT")
        sruT = res.tile([P, TC], F32R, tag="sruT")
        weT = res.tile([E, TC], F32, tag="weT")      # [e, t] combine weights
        RdT = res.tile([P, I], F32R, tag="RdT")       # [i, rd]
        dT_all = res.tile([P, (E + 1) * TC], F32R, tag="dT_all")  # [rd', e*t]
        ones1f = res.tile([1, P], F32, tag="ones1f")
        nc.gpsimd.memset(ones1f[:], 1.0)
        ones1 = res.tile([1, P], F32R, tag="ones1")
        nc.scalar.copy(ones1[:], ones1f[:])

        # ================= Phase 0: prologue =================
        with ExitStack() as pctx:
            pro = pctx.enter_context(tc.tile_pool(name="pro", bufs=1))
            pst = pctx.enter_context(tc.tile_pool(name="pro_st", bufs=2))
            pps = pctx.enter_context(tc.tile_pool(name="pro_ps", bufs=2,
                                                  space="PSUM"))
            pp1 = pctx.enter_context(tc.tile_pool(name="pro_ps1", bufs=1,
                                                  space="PSUM"))

            # x shard -> xT, token-tile-major: [h(part), t*H + hc*P + hh]
            xT = pro.tile([P, TCN * H], F32R, tag="xT")
            for t in range(TCN):
                xs = pst.tile([P, H], F32, tag="xs")
                nc.sync.dma_start(xs[:], ap["x"][t * P:(t + 1) * P, :])
                b.transpose_pack(
                    pps, xT[:, t * H:(t + 1) * H],
                    [xs[:, hc * P:(hc + 1) * P] for hc in range(HCN)],
                    identity)
            xT_r = xT[:].rearrange("p (t h) -> p t h", h=H)

            def xT_hc(hc):
                # [h128(part), (t, 128 tokens)] strided rhs, N = TC
                return xT_r[:, :, hc * P:(hc + 1) * P]

            # gate_w -> gate_T blocks [h, e] per hc
            gate_nat = pst.tile([E, H], F32, tag="gate_nat")
            nc.sync.dma_start(gate_nat[:], ap["gate_w"][:])
            gate_T = pro.tile([P, HCN * E], F32R, tag="gate_T")
            b.transpose_pack(
                pps, gate_T,
                [gate_nat[:, hc * P:(hc + 1) * P] for hc in range(HCN)],
                identity)

            # Rd -> RdT [i, rd]
            rd_nat = pst.tile([P, H], F32, tag="r_nat")
            nc.sync.dma_start(rd_nat[:, :I], ap["Rd"][:])
            b.transpose_pack(
                pps, RdT,
                [rd_nat[:, ic * P:(ic + 1) * P] for ic in range(ICN)],
                identity)

            # R projections, streamed: transpose then accumulate rg et al.
            for name, dstT in (("Rg", rgT), ("Ru", ruT),
                               ("s_Rg", srgT), ("s_Ru", sruT)):
                nat = pst.tile([P, H], F32, tag="r_nat")
                nc.sync.dma_start(nat[:], ap[name][:])
                rt = pst.tile([P, H], F32R, tag="rT")
                b.transpose_pack(
                    pps, rt,
                    [nat[:, hc * P:(hc + 1) * P] for hc in range(HCN)],
                    identity)
                acc = pp1.tile([P, TC], F32, tag="acc")
                for hc in range(HCN):
                    nc.tensor.matmul(
                        acc[:], rt[:, hc * P:(hc + 1) * P],
                        xT_hc(hc),
                        start=(hc == 0), stop=(hc == HCN - 1))
                b.copy(dstT[:], acc[:])

            # gate logits + top-4 combine weights per token tile
            zeros = pro.tile([P, E], F32, tag="zeros")
            nc.gpsimd.memset(zeros[:], 0.0)
            we_sb = []
            for t in range(TCN):
                lg = pp1.tile([P, E], F32, tag="lg")
                for hc in range(HCN):
                    nc.tensor.matmul(
                        lg[:], xT_r[:, t, hc * P:(hc + 1) * P],
                        gate_T[:, hc * E:(hc + 1) * E],
                        start=(hc == 0), stop=(hc == HCN - 1))
                nmax = pro.tile([P, 1], F32, tag="nmax")
                nc.vector.reduce_max(nmax[:], lg[:], AX.X, negate=True)
                p = pro.tile([P, E], F32, tag=f"p{t}")
                nc.scalar.activation(p[:], lg[:], AF.Exp, bias=nmax[:])
                pc = pro.tile([P, E], F32, tag="pc")
                nc.vector.tensor_copy(pc[:], p[:])
                mx = pro.tile([P, 1], F32, tag="mx")
                msk = pro.tile([P, E], F32, tag="msk")
                msk_i = pro.tile([P, E], mybir.dt.uint8, tag="msk_i")
                for _ in range(3):
                    nc.vector.reduce_max(mx[:], pc[:], AX.X)
                    nc.vector.tensor_scalar(msk_i[:], pc[:], mx[:], None,
                                            ALU.is_equal)
                    nc.vector.copy_predicated(pc[:], msk_i[:], zeros[:])
                nc.vector.reduce_max(mx[:], pc[:], AX.X)  # 4th largest
                nc.vector.tensor_scalar(msk[:], p[:], mx[:], None, ALU.is_ge)
                wu = pro.tile([P, E], F32, tag="wu")
                nc.vector.tensor_tensor(wu[:], p[:], msk[:], ALU.mult)
                den = pro.tile([P, 1], F32, tag="den")
                nc.vector.reduce_sum(den[:], wu[:], AX.X)
                rec = pro.tile([P, 1], F32, tag="rec")
                nc.vector.reciprocal(rec[:], den[:])
                we_t = pro.tile([P, E], F32, tag=f"we{t}")
                nc.vector.tensor_scalar(we_t[:], wu[:], rec[:], None, ALU.mult)
                we_sb.append(we_t)
            b.transpose_pack(pps, weT, [w[:] for w in we_sb], identity)
            nc.sync.dma_start(we_dram[:], weT[:])

        # ================= Phase 1: shared experts =================
        with ExitStack() as sctx:
            ssb = sctx.enter_context(tc.tile_pool(name="sh_sb", bufs=1))
            sst = sctx.enter_context(tc.tile_pool(name="sh_st", bufs=2))
            sps = sctx.enter_context(tc.tile_pool(name="sh_ps", bufs=2,
                                                  space="PSUM"))
            sp1 = sctx.enter_context(tc.tile_pool(name="sh_ps1", bufs=1,
                                                  space="PSUM"))

            sut_st = sst.tile([P, 3 * P], F32, tag="su_st")
            for j, name in enumerate(("s_Ug", "s_Uu", "s_Ud")):
                nc.sync.dma_start(sut_st[:, j * P:(j + 1) * P], ap[name][:])
            sUT = ssb.tile([P, 3 * P], F32R, tag="sUT")
            b.transpose_pack(sps, sUT,
                             [sut_st[:, j * P:(j + 1) * P] for j in range(3)],
                             identity)

            s_CdT = res.tile([P, H], F32R, tag="s_CdT")
            wT = {}
            for name, blocks in (("s_Cg", SICN), ("s_Cu", SICN),
                                 ("s_Rd", SICN), ("s_Cd", HCN)):
                st = sst.tile([P, SICN * P], F32, tag="s_wide_st")
                if name == "s_Rd":
                    nc.sync.dma_start(st[:, :SI], ap[name][:])
                else:
                    nc.sync.dma_start(
                        st[:, :blocks * P].rearrange("p (n r) -> p n r", r=P),
                        ap[name].rearrange("(n p) r -> p n r", p=P))
                srcs = [st[:, ic * P:(ic + 1) * P] for ic in range(blocks)]
                if name == "s_Cd":
                    wT[name] = s_CdT
                else:
                    wT[name] = ssb.tile([P, blocks * P], F32R, tag=f"{name}_T",
                                        name=f"{name}_T")
                b.transpose_pack(sps, wT[name], srcs, identity)

            gp_ps = sp1.tile([P, TC], F32, tag="gp")
            nc.tensor.matmul(gp_ps[:], sUT[:, 0:P], srgT[:])
            g_pre = ssb.tile([P, TC], F32R, tag="g_pre")
            nc.scalar.copy(g_pre[:], gp_ps[:])
            up_ps = sp1.tile([P, TC], F32, tag="gp")
            nc.tensor.matmul(up_ps[:], sUT[:, P:2 * P], sruT[:])
            u_pre = ssb.tile([P, TC], F32R, tag="u_pre")
            nc.vector.tensor_copy(u_pre[:], up_ps[:])

            rd_ps = sp1.tile([P, TC], F32, tag="rd")
            for ic in range(SICN):
                g_ps = sps.tile([P, TC], F32, tag="g")
                nc.tensor.matmul(g_ps[:], wT["s_Cg"][:, ic * P:(ic + 1) * P],
                                 g_pre[:])
                u_ps = sps.tile([P, TC], F32, tag="u")
                nc.tensor.matmul(u_ps[:], wT["s_Cu"][:, ic * P:(ic + 1) * P],
                                 u_pre[:])
                g_sil = sst.tile([P, TC], F32, tag="g_sil")
                nc.scalar.activation(g_sil[:], g_ps[:], AF.Silu)
                m = sst.tile([P, TC], F32R, tag="m")
                nc.vector.tensor_tensor(m[:], g_sil[:], u_ps[:], ALU.mult)
                nc.tensor.matmul(rd_ps[:], wT["s_Rd"][:, ic * P:(ic + 1) * P],
                                 m[:], start=(ic == 0), stop=(ic == SICN - 1))
            rd_sb = ssb.tile([P, TC], F32R, tag="rd_sb")
            b.copy(rd_sb[:], rd_ps[:])
            dT_ps = sp1.tile([P, TC], F32, tag="rd")
            nc.tensor.matmul(dT_ps[:], sUT[:, 2 * P:3 * P], rd_sb[:])
            b.copy(dT_all[:, E * TC:(E + 1) * TC], dT_ps[:])

        # ================= Phase 2: routed experts =================
        with ExitStack() as ectx:
            est = ectx.enter_context(tc.tile_pool(name="ex_st", bufs=2))
            es3 = ectx.enter_context(tc.tile_pool(name="ex_st3", bufs=3))
            eps = ectx.enter_context(tc.tile_pool(name="ex_ps", bufs=2,
                                                  space="PSUM"))
            ep1 = ectx.enter_context(tc.tile_pool(name="ex_ps1", bufs=1,
                                                  space="PSUM"))

            for e in range(E):
                u_st = est.tile([P, 3 * P], F32, tag="u_st")
                for j, name in enumerate(("Ug", "Uu", "Ud")):
                    nc.sync.dma_start(u_st[:, j * P:(j + 1) * P], ap[name][e])
                uT = est.tile([P, 3 * P], F32R, tag="uT")
                b.transpose_pack(eps, uT,
                                 [u_st[:, j * P:(j + 1) * P] for j in range(3)],
                                 identity)

                cg_st = es3.tile([P, I], F32, tag="cg_st")
                nc.sync.dma_start(
                    cg_st[:].rearrange("p (n r) -> p n r", r=P),
                    ap["Cg"][e].rearrange("(n p) r -> p n r", p=P))
                cgT = est.tile([P, I], F32R, tag="cgT")
                b.transpose_pack(eps, cgT,
                                 [cg_st[:, ic * P:(ic + 1) * P]
                                  for ic in range(ICN)], identity)
                cu_st = est.tile([P, I], F32, tag="cu_st")
                nc.sync.dma_start(
                    cu_st[:].rearrange("p (n r) -> p n r", r=P),
                    ap["Cu"][e].rearrange("(n p) r -> p n r", p=P))
                cuT = est.tile([P, I], F32R, tag="cuT")
                b.transpose_pack(eps, cuT,
                                 [cu_st[:, ic * P:(ic + 1) * P]
                                  for ic in range(ICN)], identity)

                gp_ps = ep1.tile([P, TC], F32, tag="gp")
                nc.tensor.matmul(gp_ps[:], uT[:, 0:P], rgT[:])
                g_pre = est.tile([P, TC], F32R, tag="g_pre")
                nc.scalar.copy(g_pre[:], gp_ps[:])
                # broadcast this expert's combine-weight row to all
                # partitions via a K=1 matmul with a ones column
                wrow = est.tile([1, TC], F32R, tag="wrow")
                nc.gpsimd.dma_start(wrow[:], we_dram[e:e + 1, :])
                wb_ps = ep1.tile([P, TC], F32, tag="rd", name="wb_ps")
                nc.tensor.matmul(wb_ps[:], ones1[:], wrow[:])
                wb_sb = est.tile([P, TC], F32, tag="wb_sb")
                nc.scalar.copy(wb_sb[:], wb_ps[:])
                up_ps = ep1.tile([P, TC], F32, tag="gp")
                nc.tensor.matmul(up_ps[:], uT[:, P:2 * P], ruT[:])
                # fold the routed combine weight into the u path
                u_pre = est.tile([P, TC], F32R, tag="u_pre")
                nc.vector.tensor_tensor(
                    u_pre[:], up_ps[:], wb_sb[:], ALU.mult)

                rd_ps = ep1.tile([P, TC], F32, tag="rd")
                for ic in range(ICN):
                    g_ps = eps.tile([P, TC], F32, tag="g")
                    nc.tensor.matmul(g_ps[:], cgT[:, ic * P:(ic + 1) * P],
                                     g_pre[:])
                    u_ps = eps.tile([P, TC], F32, tag="u")
                    nc.tensor.matmul(u_ps[:], cuT[:, ic * P:(ic + 1) * P],
                                     u_pre[:])
                    g_sil = est.tile([P, TC], F32, tag="g_sil")
                    nc.scalar.activation(g_sil[:], g_ps[:], AF.Silu)
                    m = est.tile([P, TC], F32R, tag="m")
                    nc.vector.tensor_tensor(m[:], g_sil[:], u_ps[:], ALU.mult)
                    nc.tensor.matmul(rd_ps[:], RdT[:, ic * P:(ic + 1) * P],
                                     m[:], start=(ic == 0),
                                     stop=(ic == ICN - 1))
                rd_sb = est.tile([P, TC], F32R, tag="rd_sb")
                b.copy(rd_sb[:], rd_ps[:])
                dT_ps = ep1.tile([P, TC], F32, tag="rd")
                nc.tensor.matmul(dT_ps[:], uT[:, 2 * P:3 * P], rd_sb[:])
                b.copy(dT_all[:, e * TC:(e + 1) * TC], dT_ps[:])

                # Pre-transpose this expert's Cd into DRAM so the final
                # phase is a dense matmul stream (keeps the PE clock warm).
                cd_st = est.tile([P, H], F32, tag="cd_st")
                nc.sync.dma_start(
                    cd_st[:].rearrange("p (n r) -> p n r", r=P),
                    ap["Cd"][e].rearrange("(n p) r -> p n r", p=P))
                cdT_stage = est.tile([P, H], F32R, tag="cdT_stage")
                b.transpose_pack(eps, cdT_stage,
                                 [cd_st[:, hc * P:(hc + 1) * P]
                                  for hc in range(HCN)], identity)
                nc.sync.dma_start(
                    cdT_dram[e].rearrange("n p r -> p n r"),
                    cdT_stage[:].rearrange("p (n r) -> p n r", r=P))

        # ================= Phase 3: down-projection =================
        with ExitStack() as fctx:
            fsb = fctx.enter_context(tc.tile_pool(name="fi_sb", bufs=1))
            fst = fctx.enter_context(tc.tile_pool(name="fi_st", bufs=4))
            fp2 = fctx.enter_context(tc.tile_pool(name="fi_st2", bufs=3))
            fps = fctx.enter_context(tc.tile_pool(name="fi_ps", bufs=2,
                                                  space="PSUM"))

            y_sb = [fsb.tile([P, H], F32, tag=f"y_sb{t}", name=f"y_sb{t}")
                    for t in range(TCN)]
            for hc in range(HCN):
                cdT_all = fp2.tile([P, E * P], F32R, tag="cdT_all")
                nc.sync.dma_start(
                    cdT_all[:].rearrange("p (n r) -> p n r", r=P),
                    cdT_dram[:, hc].rearrange("n p r -> p n r"))
                y_ps = fps.tile([P, TC], F32, tag="y")
                for j in range(E + 1):
                    if j < E:
                        lhs = cdT_all[:, j * P:(j + 1) * P]
                    else:
                        lhs = s_CdT[:, hc * P:(hc + 1) * P]
                    nc.tensor.matmul(
                        y_ps[:], lhs, dT_all[:, j * TC:(j + 1) * TC],
                        start=(j == 0), stop=(j == E))
                y_hc = fst.tile([P, TC], F32, tag="y_hc")
                b.copy(y_hc[:], y_ps[:])
                for t in range(TCN):
                    yt = fps.tile([P, P], F32, tag="ytp")
                    nc.tensor.transpose(yt[:], y_hc[:, t * P:(t + 1) * P],
                                        identity[:])
                    b.copy(y_sb[t][:, hc * P:(hc + 1) * P], yt[:])
            for t in range(TCN):
                nc.sync.dma_start(y_dram[t * P:(t + 1) * P, :], y_sb[t][:])

    return nc


def kernel(**inputs):
    inputs = {k: np.ascontiguousarray(np.asarray(v, np.float32))
              for k, v in inputs.items()}
    x = inputs["x"].reshape(TFULL, H)

    nc = bacc.Bacc("TRN2", target_bir_lowering=False, debug=False,
                   num_devices=NCORES)
    build(nc)
    nc.finalize()

    in_maps = []
    for c in range(NCORES):
        m = {"x": x[c * TC:(c + 1) * TC]}
        for k, v in inputs.items():
            if k != "x":
                m[k] = v
        in_maps.append(m)

    trace = os.environ.get("KERNEL_TRACE", "0") == "1"
    if trace:
        _ensure_ntff_hook()
    out = run_bass_kernel_spmd(nc, in_maps, list(range(NCORES)), trace=trace)
    global LAST_EXEC_NS, LAST_RESULT
    LAST_EXEC_NS = out.exec_time_ns
    LAST_RESULT = out
    results = out.results
    y = np.concatenate([results[c]["y"] for c in range(NCORES)], axis=0)
    return y.reshape(2, TFULL // 2, H)


LAST_EXEC_NS = None


def _ensure_ntff_hook():
    """Install the axon NTFF profiling hook that the agent image's antenv
    lacks, and keep profile artifacts local (no bucket upload)."""
    import sys
    import types
    import concourse.bass_utils as bu

    bu.upload_artifacts = lambda d: f"local://{d}"
    try:
        from antenv.axon_hooks import get_axon_ntff_profile_hook  # noqa
        return
    except ImportError:
        pass
    import antenv

    mod = types.ModuleType("antenv.axon_hooks")
    _holder = {}
    mod.set_axon_ntff_profile_hook = lambda h: _holder.__setitem__("h", h)
    mod.get_axon_ntff_profile_hook = lambda: _holder.get("h")
    sys.modules["antenv.axon_hooks"] = mod
    antenv.axon_hooks = mod
    if "/root/.axon_site" not in sys.path:
        sys.path.insert(0, "/root/.axon_site")
    from trn_agent_boot.trn_boot import _ntff_profile_via_ctypes

    hook = _ntff_profile_via_ctypes("/opt/axon/libaxon_pjrt.so")
    if hook is not None:
        mod.set_axon_ntff_profile_hook(hook)
